# revision 1
# baseline (speedup 1.0000x reference)
"""Trainium2 Bass kernel for nn_AttentionOp_60988535603899.

Linear-attention (elu+1 feature map) block:
  x_proj = x @ w_in.T ; qkv = x_proj @ w_qkv.T ; per-head linear attention
  with kv-state; raw (B,H,L,D)->(B,L,H*D) reshape; out_proj; residual; RMS norm.

Sharding: 8 cores = 4 batches x 2 head-groups (8 heads each). No collectives:
each core computes full-L x_proj for its batch, qkv for its 8 heads, and the
2048 output rows (= its heads' block of the interleaved reshape).

All big matmuls run as float32r (TF32-like, 4x faster than fp32 at free-dim
512); the tiny per-head attention matmuls run bf16. Final output is fp32.
"""

import sys

for _p in ("/opt/trn_rl_repo",):
    if _p not in sys.path:
        sys.path.insert(0, _p)

import numpy as np

import concourse.bass as bass  # noqa: F401  (bass must import before tile)
import concourse.mybir as mybir
import concourse.tile as tile
from concourse import bacc
from concourse.bass_utils import run_bass_kernel_spmd
from concourse.masks import make_identity

F32 = mybir.dt.float32
F32R = mybir.dt.float32r
BF16 = mybir.dt.bfloat16
FP8 = mybir.dt.float8e4
QKV_SCALE = 16.0
ALU = mybir.AluOpType
ACTF = mybir.ActivationFunctionType

B, L, CIN, DL = 4, 4096, 512, 1024
H, DH = 16, 64
HLOC = 8                  # heads per core
ELOC = 3 * HLOC * DH      # 1536 local qkv dims
LROWS = 2048              # output rows per core
EPS = float(np.finfo(np.float32).eps)
NCORES = 8

_prog_cache = {}


def _build_body(tc, xT, xTres, w_inT, w_qkvT, w_outT, norm_w, out, w_inT_b):
    nc = tc.nc

    with (
        tc.tile_pool(name="consts", bufs=1) as consts,
        tc.tile_pool(name="dram", bufs=1, space="DRAM") as dram,
        tc.tile_pool(name="dram2", bufs=1, space="DRAM") as dram2,
    ):
        # z split per head-parity: phase 4 can start on parity-0 tiles while
        # phase 3 is still producing parity-1 rows.
        z_par0 = dram.tile([LROWS // 2, DL], BF16, name="z_par0")
        z_par1 = dram2.tile([LROWS // 2, DL], BF16, name="z_par1")

        ident = consts.tile([128, 128], BF16, name="ident")
        make_identity(nc, ident)

        w_inT_sb = consts.tile([128, 4, DL], F32R, name="w_inT_sb")
        nc.sync.dma_start(w_inT_sb[:], w_inT.rearrange("(c p) d -> p c d", p=128))
        w_inT_bf = consts.tile([128, 4, DL], BF16, name="w_inT_bf")
        nc.gpsimd.dma_start(w_inT_bf[:], w_inT_b.rearrange("(c p) d -> p c d", p=128))
        w_qkvT_sb = consts.tile([128, 8, ELOC], FP8, name="w_qkvT_sb")
        nc.gpsimd.dma_start(w_qkvT_sb[:], w_qkvT.rearrange("(c p) e -> p c e", p=128))
        w_outT_sb = consts.tile([128, 8, DL], BF16, name="w_outT_sb")

        # norm_w broadcast to all 128 partitions (stride-0 partition DMA)
        nw_sb = consts.tile([128, DL], F32, name="nw_sb")
        nc.sync.dma_start(
            nw_sb[:],
            norm_w.rearrange("(a d) -> a d", a=1).to_broadcast((128, DL)),
        )

        eps_sb = consts.tile([128, 1], F32, name="eps_sb")
        nc.vector.memset(eps_sb[:], EPS)

        # persistent across phases 2-3; kv packs head h at partition half
        # (h%2)*64, slot h//2 — matching qfT's partition layout so phase-3
        # matmul operands share a base partition.
        qfT_sb = consts.tile([128, 4, L], BF16, name="qfT_sb")
        kv_sb = consts.tile([128, 4, DH + 1], BF16, name="kv_sb")
        kv_bounce = consts.tile([64, 8, DH + 1], BF16, name="kv_bounce")
        # even heads accumulate in kv_acc_a, odd in kv_acc_b (both base
        # partition 0); a final SBUF->SBUF DMA moves the odd half to
        # partitions 64-127 of kv_sb.
        kv_acc_a = consts.tile([64, 4, DH + 1], F32, name="kv_acc_a")
        kv_acc_b = consts.tile([64, 4, DH + 1], F32, name="kv_acc_b")
        nc.vector.memset(kv_acc_a[:], 0.0)
        nc.vector.memset(kv_acc_b[:], 0.0)

        # ---------------- phases 1-2: projections + kv state ----------------
        with (
            tc.tile_pool(name="w12", bufs=3) as w12,
            tc.tile_pool(name="ps_mm", bufs=2, space="PSUM") as ps_mm,
            tc.tile_pool(name="ps_kvp", bufs=4, space="PSUM") as ps_kvp,
            tc.tile_pool(name="ps_acc", bufs=1, space="PSUM") as ps_acc,
        ):
            for lt in range(8):  # 512-token tiles
                ls_l = lt * 512
                xt = w12.tile([128, 4, 512], BF16, name="xt")
                xv = xT[:, ls_l : ls_l + 512].rearrange("(c p) l -> p c l", p=128)
                for cc in range(4):
                    if lt == 0:
                        eng = nc.sync
                    else:
                        eng = nc.sync if (lt * 4 + cc) % 2 == 0 else nc.gpsimd
                    eng.dma_start(xt[:, cc, :], xv[:, cc, :])
                xp = w12.tile([128, 8, 512], FP8, name="xp")
                for dd in range(8):
                    ps = ps_mm.tile([128, 512], F32, tag="mm", name="ps1")
                    for cc in range(4):
                        nc.tensor.matmul(
                            ps[:],
                            w_inT_bf[:, cc, dd * 128 : (dd + 1) * 128],
                            xt[:, cc, :],
                            start=(cc == 0),
                            stop=(cc == 3),
                        )
                    nc.vector.tensor_copy(xp[:, dd, :], ps[:])

                # q-projection, transposed layout [dq, l]; elu+1 -> bf16
                for qq in range(4):
                    ps = ps_mm.tile([128, 512], F32, tag="mm", name="psq")
                    for cc in range(4):
                        nc.tensor.matmul(
                            ps[:],
                            w_qkvT_sb[:, 2 * cc : 2 * cc + 2, qq * 128 : (qq + 1) * 128],
                            xp[:, 2 * cc : 2 * cc + 2, :],
                            start=(cc == 0),
                            stop=(cc == 3),
                            perf_mode=mybir.MatmulPerfMode.DoubleRow,
                        )
                    eq = w12.tile([128, 512], BF16, name="eq")
                    rq = w12.tile([128, 512], BF16, name="rq")
                    nc.scalar.activation(eq[:], ps[:], ACTF.Exp, scale=1.0 / QKV_SCALE)
                    nc.vector.tensor_scalar(rq[:], ps[:], 0.0, 1.0 / QKV_SCALE, ALU.max, ALU.mult)
                    nc.vector.tensor_scalar(eq[:], eq[:], 1.0, None, ALU.min)
                    nc.vector.tensor_tensor(
                        qfT_sb[:, qq, ls_l : ls_l + 512], eq[:], rq[:], ALU.add
                    )

                # k/v projection in [l, e] layout, 128-token subtiles
                for ls in range(4):
                    lhs = xp[:, :, ls * 128 : (ls + 1) * 128]
                    k_ps = ps_kvp.tile([128, 512], F32, tag="kvp", name="k_ps")
                    v_ps = ps_kvp.tile([128, 512], F32, tag="kvp", name="v_ps")
                    for cc in range(4):
                        nc.tensor.matmul(
                            k_ps[:],
                            lhs[:, 2 * cc : 2 * cc + 2, :],
                            w_qkvT_sb[:, 2 * cc : 2 * cc + 2, 512:1024],
                            start=(cc == 0),
                            stop=(cc == 3),
                            perf_mode=mybir.MatmulPerfMode.DoubleRow,
                        )
                    for cc in range(4):
                        nc.tensor.matmul(
                            v_ps[:],
                            lhs[:, 2 * cc : 2 * cc + 2, :],
                            w_qkvT_sb[:, 2 * cc : 2 * cc + 2, 1024:1536],
                            start=(cc == 0),
                            stop=(cc == 3),
                            perf_mode=mybir.MatmulPerfMode.DoubleRow,
                        )
                    kf = w12.tile([128, 512], BF16, name="kf")
                    ek = w12.tile([128, 512], BF16, name="ek")
                    nc.scalar.activation(ek[:], k_ps[:], ACTF.Exp, scale=1.0 / QKV_SCALE)
                    nc.vector.tensor_scalar(kf[:], k_ps[:], 0.0, 1.0 / QKV_SCALE, ALU.max, ALU.mult)
                    nc.vector.tensor_scalar(ek[:], ek[:], 1.0, None, ALU.min)
                    nc.vector.tensor_tensor(kf[:], kf[:], ek[:], ALU.add)

                    vt = w12.tile([128, HLOC, DH + 1], BF16, name="vt")
                    nc.vector.tensor_scalar(
                        vt[:, :, 0:DH],
                        v_ps[:].rearrange("p (h m) -> p h m", m=DH),
                        1.0 / QKV_SCALE,
                        None,
                        ALU.mult,
                    )
                    nc.vector.memset(vt[:, :, DH : DH + 1], 1.0)
                    kv_ps_a = ps_acc.tile([64, 4, DH + 1], F32, tag="kvpsa", name="kv_ps_a")
                    kv_ps_b = ps_acc.tile([64, 4, DH + 1], F32, tag="kvpsb", name="kv_ps_b")
                    for h in range(HLOC):
                        nc.tensor.matmul(
                            (kv_ps_a if h % 2 == 0 else kv_ps_b)[:, h // 2, :],
                            kf[:, h * DH : (h + 1) * DH],
                            vt[:, h, :],
                            start=True,
                            stop=True,
                        )
                    nc.vector.tensor_tensor(kv_acc_a[:], kv_acc_a[:], kv_ps_a[:], ALU.add)
                    nc.vector.tensor_tensor(kv_acc_b[:], kv_acc_b[:], kv_ps_b[:], ALU.add)

            # cast to bf16 (same partitions), then partition-move via DMA
            nc.vector.tensor_copy(kv_bounce[:, 0:4, :], kv_acc_a[:])
            nc.vector.tensor_copy(kv_bounce[:, 4:8, :], kv_acc_b[:])
            nc.sync.dma_start(kv_sb[0:64, :, :], kv_bounce[:, 0:4, :])
            nc.sync.dma_start(kv_sb[64:128, :, :], kv_bounce[:, 4:8, :])

        nc.gpsimd.dma_start(w_outT_sb[:], w_outT.rearrange("(c p) d -> p c d", p=128))

        # ---------------- phase 3: attention out + reshape to z ----------------
        # Head h = 2*s + par lives at partitions par*64..+64, slot s of
        # qfT_sb / kv_sb. Each PSUM bank sees a single input base partition
        # (mixing tile_position row offsets within one bank hangs the device).
        with (
            tc.tile_pool(name="p3", bufs=6) as p3,
            tc.tile_pool(name="ps3", bufs=6, space="PSUM") as ps3p,
        ):
            for par in range(2):
                p0 = par * 64
                zp = (z_par0 if par == 0 else z_par1)
                zv = zp.rearrange("(s rr) (j d) -> rr j s d", rr=256, d=DH)
                for lt in range(32):  # 128-token tiles
                    ps3 = ps3p.tile([128, 4, DH + 1], F32, tag="att", name="ps3")
                    for s in range(4):
                        nc.tensor.matmul(
                            ps3[:, s, :],
                            qfT_sb[p0 : p0 + 64, s, lt * 128 : (lt + 1) * 128],
                            kv_sb[p0 : p0 + 64, s, :],
                            start=True,
                            stop=True,
                        )
                    rec = p3.tile([128, 4], F32, name="rec")
                    nc.vector.reciprocal(rec[:], ps3[:, :, DH])
                    att = p3.tile([128, 4, DH], BF16, name="att")
                    nc.vector.tensor_tensor(
                        att[:],
                        ps3[:, :, 0:DH],
                        rec[:, :, None].to_broadcast((128, 4, DH)),
                        ALU.mult,
                    )
                    eng = nc.gpsimd if lt % 2 == 0 else nc.sync
                    eng.dma_start(zv[lt * 8 : (lt + 1) * 8], att[:])

        # ---------------- phase 4: out_proj + residual + RMS norm ----------------
        # Tile order: parity-0 heads first (their z rows finish first).
        with (
            tc.tile_pool(name="p4", bufs=4) as p4,
            tc.tile_pool(name="p4z", bufs=4) as p4z,
            tc.tile_pool(name="psT", bufs=2, space="PSUM") as psT,
            tc.tile_pool(name="ps4", bufs=3, space="PSUM") as ps4p,
        ):
            for par in range(2):
                zp = (z_par0 if par == 0 else z_par1)
                for s in range(4):
                    for half in range(2):
                        h = 2 * s + par
                        zt = h * 2 + half          # output row block index
                        zr = zt * 128
                        zpr = s * 256 + half * 128  # row offset inside zp
                        zt_sb = p4.tile([128, DL], BF16, name="zt_sb")
                        nc.sync.dma_start(zt_sb[:], zp[zpr : zpr + 128, :])
                        xr = p4.tile([128, 4, 128], F32R, name="xr")
                        nc.sync.dma_start(
                            xr[:],
                            xTres[:, zr : zr + 128].rearrange(
                                "(c p) l -> p c l", p=128
                            ),
                        )
                        # out_proj and the recomputed x_proj residual share one
                        # PSUM group: y = z @ w_out.T + x_row @ w_in.T
                        ps4 = ps4p.tile([128, DL], F32, name="ps4")
                        for cc in range(8):
                            tp = psT.tile([128, 128], BF16, tag="tp", name="tp")
                            nc.tensor.transpose(
                                tp[:], zt_sb[:, cc * 128 : (cc + 1) * 128], ident[:]
                            )
                            zTc = p4z.tile([128, 128], BF16, name="zTc")
                            nc.any.tensor_copy(zTc[:], tp[:])
                            nc.tensor.matmul(
                                ps4[:, 0:512],
                                zTc[:],
                                w_outT_sb[:, cc, 0:512],
                                start=(cc == 0),
                                stop=False,
                            )
                            nc.tensor.matmul(
                                ps4[:, 512:1024],
                                zTc[:],
                                w_outT_sb[:, cc, 512:1024],
                                start=(cc == 0),
                                stop=False,
                            )
                        for cc in range(4):
                            nc.tensor.matmul(
                                ps4[:, 0:512],
                                xr[:, cc, :],
                                w_inT_sb[:, cc, 0:512],
                                start=False,
                                stop=(cc == 3),
                            )
                            nc.tensor.matmul(
                                ps4[:, 512:1024],
                                xr[:, cc, :],
                                w_inT_sb[:, cc, 512:1024],
                                start=False,
                                stop=(cc == 3),
                            )
                        # RMS stats and final scale read the PSUM directly —
                        # no SBUF copy of y needed.
                        sq = p4.tile([128, DL], F32, name="sq")
                        ssum = p4.tile([128, 1], F32, name="ssum")
                        nc.scalar.activation(sq[:], ps4[:], ACTF.Square, accum_out=ssum[:])
                        srt = p4.tile([128, 1], F32, name="srt")
                        nc.scalar.activation(
                            srt[:], ssum[:], ACTF.Sqrt, scale=1.0 / DL, bias=eps_sb[:]
                        )
                        rcp = p4.tile([128, 1], F32, name="rcp")
                        nc.vector.reciprocal(rcp[:], srt[:])
                        o = p4.tile([128, DL], F32, name="o")
                        nc.vector.tensor_scalar(o[:], ps4[:], rcp[:], None, ALU.mult)
                        nc.gpsimd.tensor_tensor(o[:], o[:], nw_sb[:], ALU.mult)
                        nc.sync.dma_start(out[zr : zr + 128, :], o[:])


def build_program():
    if "nc" in _prog_cache:
        return _prog_cache["nc"]
    nc = bacc.Bacc(None, target_bir_lowering=False, debug=False)
    xT = nc.dram_tensor("xT", [CIN, L], BF16, kind="ExternalInput")
    xTres = nc.dram_tensor("xTres", [CIN, LROWS], F32R, kind="ExternalInput")
    w_inT = nc.dram_tensor("w_inT", [CIN, DL], F32R, kind="ExternalInput")
    w_inT_b = nc.dram_tensor("w_inT_b", [CIN, DL], BF16, kind="ExternalInput")
    w_qkvT = nc.dram_tensor("w_qkvT", [DL, ELOC], FP8, kind="ExternalInput")
    w_outT = nc.dram_tensor("w_outT", [DL, DL], BF16, kind="ExternalInput")
    norm_w = nc.dram_tensor("norm_w", [DL], F32, kind="ExternalInput")
    out = nc.dram_tensor("out", [LROWS, DL], F32, kind="ExternalOutput")
    with tile.TileContext(nc) as tc:
        _build_body(tc, xT[:], xTres[:], w_inT[:], w_qkvT[:], w_outT[:], norm_w[:], out[:], w_inT_b[:])
    nc.compile()
    _prog_cache["nc"] = nc
    return nc


def make_in_maps(x, w_in, w_qkv, w_out, norm_w):
    import ml_dtypes

    bf16 = ml_dtypes.bfloat16
    f8e4 = mybir.dt.np(mybir.dt.float8e4)
    x = np.ascontiguousarray(np.asarray(x, dtype=np.float32))
    w_in = np.asarray(w_in, dtype=np.float32)
    w_qkv = np.asarray(w_qkv, dtype=np.float32)
    w_out = np.asarray(w_out, dtype=np.float32)
    norm_w = np.ascontiguousarray(np.asarray(norm_w, dtype=np.float32))
    w_inT = np.ascontiguousarray(w_in.T)
    w_outT = np.ascontiguousarray(w_out.T).astype(bf16)
    in_maps = []
    for core in range(NCORES):
        b, g = core // 2, core % 2
        sl = slice(g * 512, (g + 1) * 512)
        wq = np.concatenate([w_qkv[0:1024][sl], w_qkv[1024:2048][sl], w_qkv[2048:3072][sl]], axis=0)
        in_maps.append(
            {
                "xT": np.ascontiguousarray(x[b].T).astype(bf16),
                "xTres": np.ascontiguousarray(x[b, g * LROWS : (g + 1) * LROWS].T),
                "w_inT": w_inT,
                "w_inT_b": w_inT.astype(bf16),
                "w_qkvT": (np.ascontiguousarray(wq.T) * 16.0).astype(f8e4),
                "w_outT": w_outT,
                "norm_w": norm_w,
            }
        )
    return in_maps


def run_on_cores(in_maps, trace=False):
    nc = build_program()
    return run_bass_kernel_spmd(nc, in_maps, list(range(NCORES)), trace=trace)


def assemble(results):
    out = np.empty((B, L, DL), np.float32)
    for core in range(NCORES):
        b, g = core // 2, core % 2
        out[b, g * LROWS : (g + 1) * LROWS] = results[core]["out"]
    return out


def kernel(x, w_in, w_qkv, w_out, norm_w):
    in_maps = make_in_maps(x, w_in, w_qkv, w_out, norm_w)
    res = run_on_cores(in_maps, trace=False)
    return assemble(res.results)


if __name__ == "__main__":
    nc = build_program()
    print("program built + compiled OK")



# revision 13
# speedup vs baseline: 1.2561x; 1.2561x over previous
"""Trainium2 Bass kernel for nn_AttentionOp_60988535603899.

Linear-attention (elu+1 feature map) block:
  x_proj = x @ w_in.T ; qkv = x_proj @ w_qkv.T ; per-head linear attention
  with kv-state; raw (B,H,L,D)->(B,L,H*D) reshape; out_proj; residual; RMS norm.

Sharding: 8 cores = 4 batches x 2 head-groups (8 heads each). No collectives.

v2 design (vs baseline):
  - All projection matmuls fp8 DoubleRow (x_proj, qkv, out_proj); residual
    recompute in bf16. Scales: w_in/w_qkv/w_out/z carry x16 each; the RMS
    normalization at the end cancels the combined scale automatically.
  - The attention output is produced directly TRANSPOSED (zT[(j%2)*64+d,
    cc, m] = z[l=16m+2cc+(j%2), d]) so out_proj needs no PE transposes at
    all (the transposes poisoned HAM clock-gating in the baseline) and z
    never round-trips through DRAM.
  - The 1/normalizer is folded into qfT before the z matmul: norm row per
    head via a block-diagonal ksum matmul, reciprocal, then a PE
    broadcast-matmul (sel outer product) to spread recip across the 128
    partitions of each qfT slot.
  - kv state accumulates directly in PSUM across all 32 token chunks
    (single has_written clear at the start) - no Vector adds, which were
    stalling the PE every l-tile in the baseline and re-throttling HAM.
"""

import sys

for _p in ("/opt/trn_rl_repo",):
    if _p not in sys.path:
        sys.path.insert(0, _p)

import numpy as np

import concourse.bass as bass  # noqa: F401  (bass must import before tile)
import concourse.mybir as mybir
import concourse.tile as tile
from concourse import bacc
from concourse.bass_utils import run_bass_kernel_spmd

F32 = mybir.dt.float32
BF16 = mybir.dt.bfloat16
FP8 = mybir.dt.float8e4
ALU = mybir.AluOpType
ACTF = mybir.ActivationFunctionType
DR = mybir.MatmulPerfMode.DoubleRow

B, L, CIN, DL = 4, 4096, 512, 1024
H, DH = 16, 64
HLOC = 8                  # heads per core
ELOC = 3 * HLOC * DH      # 1536 local qkv dims
LROWS = 2048              # output rows per core
EPS = float(np.finfo(np.float32).eps)
NCORES = 8
W16 = 16.0                # fp8 weight/activation scale

_prog_cache = {}


def _build_body(tc, xT, xresd, w_inT_f8d, w_inT_bfd, w_qkvTd, w_outTd, norm_w, sel2d, out):
    nc = tc.nc

    with tc.tile_pool(name="consts", bufs=1) as consts:
        # ---------------- persistent tiles ----------------
        x_sb = consts.tile([128, 4, L], FP8, name="x_sb")
        xres = consts.tile([128, 4, LROWS], BF16, name="xres")
        w_inT_f8 = consts.tile([128, 4, DL], FP8, name="w_inT_f8")
        w_inT_bf = consts.tile([128, 4, DL], BF16, name="w_inT_bf")
        w_qkvT = consts.tile([128, 8, ELOC], FP8, name="w_qkvT")
        w_outT = consts.tile([128, 8, DL], FP8, name="w_outT")
        qfT = consts.tile([128, 4, L], BF16, name="qfT")
        zT = consts.tile([128, HLOC, 8, 256], FP8, name="zT")
        kv_sb = consts.tile([128, 4, DH + 1], BF16, name="kv_sb")
        kv_bounce = consts.tile([64, 8, DH + 1], BF16, name="kv_bounce")
        ksb = consts.tile([128, 4, 2], BF16, name="ksb")
        sel2 = consts.tile([2, 128], BF16, name="sel2")
        nw_sb = consts.tile([128, DL], F32, name="nw_sb")
        eps_sb = consts.tile([128, 1], F32, name="eps_sb")

        # ---------------- input DMAs ----------------
        # sync queue: x chunks (needed first); gpsimd: weights.
        nc.gpsimd.dma_start(
            w_inT_f8[:], w_inT_f8d.rearrange("(c p) d -> p c d", p=128)
        )
        xv = xT.rearrange("(c p) l -> p c l", p=128)
        for i in range(4):
            eng = nc.sync if i % 2 == 0 else nc.gpsimd
            eng.dma_start(x_sb[:, :, i * 1024 : (i + 1) * 1024], xv[:, :, i * 1024 : (i + 1) * 1024])
        nc.gpsimd.dma_start(w_qkvT[:], w_qkvTd.rearrange("(c p) e -> p c e", p=128))
        nc.sync.dma_start(
            nw_sb[:],
            norm_w.rearrange("(a d) -> a d", a=1).to_broadcast((128, DL)),
        )
        nc.gpsimd.dma_start(w_outT[:], w_outTd.rearrange("(c p) e -> p c e", p=128))
        nc.gpsimd.dma_start(w_inT_bf[:], w_inT_bfd.rearrange("(c p) d -> p c d", p=128))
        nc.sync.dma_start(xres[:], xresd.rearrange("(c p) l -> p c l", p=128))

        nc.vector.memset(eps_sb[:], EPS)
        # sel2[i, p] = 1 iff p//64 == i (partition-broadcast selector)
        nc.sync.dma_start(sel2[:], sel2d[:])

        # ---------------- phases 1-2: projections + kv state ----------------
        with (
            tc.tile_pool(name="w12", bufs=3) as w12,
            tc.tile_pool(name="ps_x", bufs=2, space="PSUM") as ps_x,
            tc.tile_pool(name="ps_kv", bufs=4, space="PSUM") as ps_kv,
            tc.tile_pool(name="ps_acc", bufs=1, space="PSUM") as ps_acc,
        ):
            # even heads accumulate in kv_ps_a, odd in kv_ps_b, across all
            # 32 token chunks (PSUM has_written semantics: one bank clear
            # at the start, then per-element accumulate/overwrite).
            kv_ps_a = ps_acc.tile([64, 4, DH + 1], F32, name="kv_ps_a")
            kv_ps_b = ps_acc.tile([64, 4, DH + 1], F32, name="kv_ps_b")

            for lt in range(8):  # 512-token tiles
                ls_l = lt * 512
                # x_proj -> xp (= 16*x_proj) in fp8, [dl, l] layout
                xp = w12.tile([128, 8, 512], FP8, name="xp")
                for dd in range(8):
                    ps = ps_x.tile([128, 512], F32, tag="mm", name="ps1")
                    for c2 in range(2):
                        nc.tensor.matmul(
                            ps[:],
                            w_inT_f8[:, 2 * c2 : 2 * c2 + 2, dd * 128 : (dd + 1) * 128],
                            x_sb[:, 2 * c2 : 2 * c2 + 2, ls_l : ls_l + 512],
                            start=(c2 == 0),
                            stop=(c2 == 1),
                            perf_mode=DR,
                        )
                    nc.any.tensor_copy(xp[:, dd, :], ps[:])

                # q-projection, transposed layout [dq, l]; elu+1 -> bf16
                for qq in range(4):
                    ps = ps_x.tile([128, 512], F32, tag="mm", name="psq")
                    for cc in range(4):
                        nc.tensor.matmul(
                            ps[:],
                            w_qkvT[:, 2 * cc : 2 * cc + 2, qq * 128 : (qq + 1) * 128],
                            xp[:, 2 * cc : 2 * cc + 2, :],
                            start=(cc == 0),
                            stop=(cc == 3),
                            perf_mode=DR,
                        )
                    eq = w12.tile([128, 512], BF16, name="eq")
                    rq = w12.tile([128, 512], BF16, name="rq")
                    nc.scalar.activation(eq[:], ps[:], ACTF.Exp, scale=1.0 / 256.0)
                    nc.vector.tensor_scalar(rq[:], ps[:], 0.0, 1.0 / 256.0, ALU.max, ALU.mult)
                    nc.vector.tensor_scalar(eq[:], eq[:], 1.0, None, ALU.min)
                    nc.vector.tensor_tensor(
                        qfT[:, qq, ls_l : ls_l + 512], eq[:], rq[:], ALU.add
                    )

                # k/v projection in [l, e] layout, 128-token subtiles
                for ls in range(4):
                    lhs = xp[:, :, ls * 128 : (ls + 1) * 128]
                    k_ps = ps_kv.tile([128, 512], F32, tag="kvp", name="k_ps")
                    v_ps = ps_kv.tile([128, 512], F32, tag="kvp", name="v_ps")
                    for cc in range(4):
                        nc.tensor.matmul(
                            k_ps[:],
                            lhs[:, 2 * cc : 2 * cc + 2, :],
                            w_qkvT[:, 2 * cc : 2 * cc + 2, 512:1024],
                            start=(cc == 0),
                            stop=(cc == 3),
                            perf_mode=DR,
                        )
                    for cc in range(4):
                        nc.tensor.matmul(
                            v_ps[:],
                            lhs[:, 2 * cc : 2 * cc + 2, :],
                            w_qkvT[:, 2 * cc : 2 * cc + 2, 1024:1536],
                            start=(cc == 0),
                            stop=(cc == 3),
                            perf_mode=DR,
                        )
                    kf = w12.tile([128, 512], BF16, name="kf")
                    ek = w12.tile([128, 512], BF16, name="ek")
                    nc.scalar.activation(ek[:], k_ps[:], ACTF.Exp, scale=1.0 / 256.0)
                    nc.vector.tensor_scalar(kf[:], k_ps[:], 0.0, 1.0 / 256.0, ALU.max, ALU.mult)
                    nc.vector.tensor_scalar(ek[:], ek[:], 1.0, None, ALU.min)
                    nc.vector.tensor_tensor(kf[:], kf[:], ek[:], ALU.add)

                    vt = w12.tile([128, HLOC, DH + 1], BF16, name="vt")
                    nc.vector.tensor_scalar(
                        vt[:, :, 0:DH],
                        v_ps[:].rearrange("p (h m) -> p h m", m=DH),
                        1.0 / 256.0,
                        None,
                        ALU.mult,
                    )
                    nc.vector.memset(vt[:, :, DH : DH + 1], 1.0)
                    first = lt == 0 and ls == 0
                    last = lt == 7 and ls == 3
                    for h in range(HLOC):
                        nc.tensor.matmul(
                            (kv_ps_a if h % 2 == 0 else kv_ps_b)[:, h // 2, :],
                            kf[:, h * DH : (h + 1) * DH],
                            vt[:, h, :],
                            start=(first and h < 2),
                            stop=(last and h >= 6),
                            skip_group_check=True,
                        )

            # kv state: cast to bf16, partition-move odd heads to 64..127
            nc.vector.tensor_copy(kv_bounce[:, 0:4, :], kv_ps_a[:])
            nc.vector.tensor_copy(kv_bounce[:, 4:8, :], kv_ps_b[:])
            nc.sync.dma_start(kv_sb[0:64, :, :], kv_bounce[:, 0:4, :])
            nc.sync.dma_start(kv_sb[64:128, :, :], kv_bounce[:, 4:8, :])

        # ---------------- normalizer: norm rows, recip, fold into qfT ----
        # ksb[64*i+d, s, i] = ksum of head 2s+i (kv col DH), else 0.
        nc.vector.memset(ksb[:], 0.0)
        nc.vector.tensor_copy(ksb[0:64, :, 0:1], kv_sb[0:64, :, DH : DH + 1])
        nc.vector.tensor_copy(ksb[64:128, :, 1:2], kv_sb[64:128, :, DH : DH + 1])

        # recipT_c = 1/norm; the x16 z-scale is folded into sel2 (=16.0).
        with (
            tc.tile_pool(name="nsb", bufs=2) as nsb,
            tc.tile_pool(name="ps_n", bufs=4, space="PSUM") as ps_n,
            tc.tile_pool(name="ps_rb", bufs=2, space="PSUM") as ps_rb,
        ):
            for c in range(8):
                cs = slice(c * 512, (c + 1) * 512)
                recipT = nsb.tile([2, 4, 512], BF16, name="recipT")
                for s in range(4):
                    pn = ps_n.tile([2, 512], F32, tag="n", name="pn")
                    nc.tensor.matmul(
                        pn[:], ksb[:, s, :], qfT[:, s, cs], start=True, stop=True
                    )
                    with nc.allow_low_precision(reason="recip of large norms"):
                        nc.vector.reciprocal(recipT[:, s, :], pn[:])
                for s in range(4):
                    rb = ps_rb.tile([128, 512], F32, tag="rb", name="rb")
                    nc.tensor.matmul(
                        rb[:], sel2[:], recipT[:, s, :], start=True, stop=True
                    )
                    nc.vector.tensor_tensor(qfT[:, s, cs], qfT[:, s, cs], rb[:], ALU.mult)

        # ---------------- phases 3-4: zT + out_proj + residual + RMS ------
        with (
            tc.tile_pool(name="p34", bufs=3) as p34,
            tc.tile_pool(name="ps3", bufs=4, space="PSUM") as ps3p,
            tc.tile_pool(name="ps4", bufs=2, space="PSUM") as ps4p,
        ):

            def phase4(h):
                s, par = h // 2, h % 2
                for half in range(2):
                    lr = h * 256 + half * 128
                    ps4 = ps4p.tile([128, DL], F32, tag="p4", name="ps4")
                    for e in range(2):
                        es = slice(e * 512, (e + 1) * 512)
                        for t in range(4):
                            nc.tensor.matmul(
                                ps4[:, es],
                                zT[:, h, 2 * t : 2 * t + 2, half * 128 : (half + 1) * 128],
                                w_outT[:, 2 * t : 2 * t + 2, es],
                                start=(t == 0),
                                stop=False,
                                perf_mode=DR,
                            )
                        for cc in range(4):
                            nc.tensor.matmul(
                                ps4[:, es],
                                xres[:, cc, lr : lr + 128],
                                w_inT_bf[:, cc, es],
                                start=False,
                                stop=(cc == 3),
                            )
                    # RMS stats + final scale read PSUM directly
                    sq = p34.tile([128, DL], F32, name="sq")
                    ssum = p34.tile([128, 1], F32, name="ssum")
                    nc.scalar.activation(sq[:], ps4[:], ACTF.Square, accum_out=ssum[:])
                    srt = p34.tile([128, 1], F32, name="srt")
                    nc.scalar.activation(
                        srt[:], ssum[:], ACTF.Sqrt, scale=1.0 / DL, bias=eps_sb[:]
                    )
                    rcp = p34.tile([128, 1], F32, name="rcp")
                    nc.vector.reciprocal(rcp[:], srt[:])
                    o = p34.tile([128, DL], F32, name="o")
                    nc.vector.tensor_scalar(o[:], ps4[:], rcp[:], None, ALU.mult)
                    nc.gpsimd.tensor_tensor(o[:], o[:], nw_sb[:], ALU.mult)
                    eng = nc.sync if (h + half) % 2 == 0 else nc.gpsimd
                    eng.dma_start(out[lr : lr + 128, :], o[:])

            for h in range(HLOC):
                s, par = h // 2, h % 2
                p0 = par * 64
                # phase 3: zT[p, cc, m] = z[l=16m+2cc+(p//64), d=p%64] * 16
                for cc in range(8):
                    ps3 = ps3p.tile([128, 256], F32, tag="p3", name="ps3")
                    for pj in range(2):
                        nc.tensor.matmul(
                            ps3[pj * 64 : (pj + 1) * 64, :],
                            kv_sb[p0 : p0 + 64, s, 0:DH],
                            qfT[p0 : p0 + 64, s, 2 * cc + pj : L : 16],
                            start=True,
                            stop=True,
                        )
                    nc.any.tensor_copy(zT[:, h, cc, :], ps3[:])
                if h > 0:
                    phase4(h - 1)
            phase4(HLOC - 1)


def build_program():
    if "nc" in _prog_cache:
        return _prog_cache["nc"]
    nc = bacc.Bacc(None, target_bir_lowering=False, debug=False)
    xT = nc.dram_tensor("xT", [CIN, L], FP8, kind="ExternalInput")
    xresd = nc.dram_tensor("xres", [CIN, LROWS], BF16, kind="ExternalInput")
    w_inT_f8d = nc.dram_tensor("w_inT_f8", [CIN, DL], FP8, kind="ExternalInput")
    w_inT_bfd = nc.dram_tensor("w_inT_bf", [CIN, DL], BF16, kind="ExternalInput")
    w_qkvTd = nc.dram_tensor("w_qkvT", [DL, ELOC], FP8, kind="ExternalInput")
    w_outTd = nc.dram_tensor("w_outT", [DL, DL], FP8, kind="ExternalInput")
    norm_w = nc.dram_tensor("norm_w", [DL], F32, kind="ExternalInput")
    sel2d = nc.dram_tensor("sel2", [2, 128], BF16, kind="ExternalInput")
    out = nc.dram_tensor("out", [LROWS, DL], F32, kind="ExternalOutput")
    with tile.TileContext(nc) as tc:
        _build_body(
            tc, xT[:], xresd[:], w_inT_f8d[:], w_inT_bfd[:], w_qkvTd[:],
            w_outTd[:], norm_w[:], sel2d[:], out[:],
        )
    nc.compile()
    _prog_cache["nc"] = nc
    return nc


def make_in_maps(x, w_in, w_qkv, w_out, norm_w):
    import ml_dtypes

    bf16 = ml_dtypes.bfloat16
    f8e4 = mybir.dt.np(mybir.dt.float8e4)
    x = np.ascontiguousarray(np.asarray(x, dtype=np.float32))
    w_in = np.asarray(w_in, dtype=np.float32)
    w_qkv = np.asarray(w_qkv, dtype=np.float32)
    w_out = np.asarray(w_out, dtype=np.float32)
    norm_w = np.ascontiguousarray(np.asarray(norm_w, dtype=np.float32))

    w_inT_f8 = np.ascontiguousarray(w_in.T * W16).astype(f8e4)
    w_inT_bf = np.ascontiguousarray(w_in.T * (W16 * W16)).astype(bf16)
    # w_outT in (token-parity, d) partition layout, cc-chunked:
    # arr[p, cc, e] = w_out.T[(2cc + p//64)*64 + p%64, e] * 16
    wt = np.ascontiguousarray(w_out.T).reshape(8, 2, 64, DL)
    w_outT = np.ascontiguousarray(
        (wt.transpose(1, 2, 0, 3).reshape(128, 8, DL) * W16)
        .transpose(1, 0, 2)
        .reshape(DL, DL)
    ).astype(f8e4)

    sel2 = np.zeros((2, 128), dtype=bf16)
    sel2[0, 0:64] = W16   # broadcast + x16 z-scale in one
    sel2[1, 64:128] = W16

    in_maps = []
    for core in range(NCORES):
        b, g = core // 2, core % 2
        sl = slice(g * 512, (g + 1) * 512)
        wq = np.concatenate(
            [w_qkv[0:1024][sl], w_qkv[1024:2048][sl], w_qkv[2048:3072][sl]], axis=0
        )
        in_maps.append(
            {
                "xT": np.ascontiguousarray(x[b].T).astype(f8e4),
                "xres": np.ascontiguousarray(
                    x[b, g * LROWS : (g + 1) * LROWS].T
                ).astype(bf16),
                "w_inT_f8": w_inT_f8,
                "w_inT_bf": w_inT_bf,
                "w_qkvT": (np.ascontiguousarray(wq.T) * W16).astype(f8e4),
                "w_outT": w_outT,
                "norm_w": norm_w,
                "sel2": sel2,
            }
        )
    return in_maps


def run_on_cores(in_maps, trace=False):
    nc = build_program()
    return run_bass_kernel_spmd(nc, in_maps, list(range(NCORES)), trace=trace)


def assemble(results):
    out = np.empty((B, L, DL), np.float32)
    for core in range(NCORES):
        b, g = core // 2, core % 2
        out[b, g * LROWS : (g + 1) * LROWS] = results[core]["out"]
    return out


def kernel(x, w_in, w_qkv, w_out, norm_w):
    in_maps = make_in_maps(x, w_in, w_qkv, w_out, norm_w)
    res = run_on_cores(in_maps, trace=False)
    return assemble(res.results)


if __name__ == "__main__":
    nc = build_program()
    print("program built + compiled OK")


# revision 24
# speedup vs baseline: 1.4388x; 1.1454x over previous
"""Trainium2 Bass kernel for nn_AttentionOp_60988535603899.

Linear-attention (elu+1 feature map) block:
  x_proj = x @ w_in.T ; qkv = x_proj @ w_qkv.T ; per-head linear attention
  with kv-state; raw (B,H,L,D)->(B,L,H*D) reshape; out_proj; residual; RMS norm.

Sharding: 8 cores = 4 batches x 2 head-groups (8 heads each). No collectives.

v2 design (vs baseline):
  - All projection matmuls fp8 DoubleRow (x_proj, qkv, out_proj); residual
    recompute in bf16. Scales: w_in/w_qkv/w_out/z carry x16 each; the RMS
    normalization at the end cancels the combined scale automatically.
  - The attention output is produced directly TRANSPOSED (zT[(j%2)*64+d,
    cc, m] = z[l=16m+2cc+(j%2), d]) so out_proj needs no PE transposes at
    all (the transposes poisoned HAM clock-gating in the baseline) and z
    never round-trips through DRAM.
  - The 1/normalizer is folded into qfT before the z matmul: norm row per
    head via a block-diagonal ksum matmul, reciprocal, then a PE
    broadcast-matmul (sel outer product) to spread recip across the 128
    partitions of each qfT slot.
  - kv state accumulates directly in PSUM across all 32 token chunks
    (single has_written clear at the start) - no Vector adds, which were
    stalling the PE every l-tile in the baseline and re-throttling HAM.
"""

import sys

for _p in ("/opt/trn_rl_repo",):
    if _p not in sys.path:
        sys.path.insert(0, _p)

import numpy as np

import concourse.bass as bass  # noqa: F401  (bass must import before tile)
import concourse.mybir as mybir
import concourse.tile as tile
from concourse import bacc
from concourse.bass_utils import run_bass_kernel_spmd

F32 = mybir.dt.float32
BF16 = mybir.dt.bfloat16
FP8 = mybir.dt.float8e4
ALU = mybir.AluOpType
ACTF = mybir.ActivationFunctionType
DR = mybir.MatmulPerfMode.DoubleRow

B, L, CIN, DL = 4, 4096, 512, 1024
H, DH = 16, 64
HLOC = 8                  # heads per core
ELOC = 3 * HLOC * DH      # 1536 local qkv dims
LROWS = 2048              # output rows per core
EPS = float(np.finfo(np.float32).eps)
NCORES = 8
W16 = 16.0                # fp8 weight/activation scale

_prog_cache = {}


def _build_body(tc, xT, xresd, w_inT_f8d, w_inT_bfd, w_qkvTd, w_outTd, norm_w, sel2d, out):
    nc = tc.nc

    with tc.tile_pool(name="consts", bufs=1) as consts:
        # ---------------- persistent tiles ----------------
        x_sb = consts.tile([128, 4, L], FP8, name="x_sb")
        xres = consts.tile([128, 4, LROWS], BF16, name="xres")
        w_inT_f8 = consts.tile([128, 4, DL], FP8, name="w_inT_f8")
        w_inT_bf = consts.tile([128, 4, DL], BF16, name="w_inT_bf")
        w_qkvT = consts.tile([128, 8, ELOC], FP8, name="w_qkvT")
        w_outT = consts.tile([128, 8, DL], FP8, name="w_outT")
        qfT = consts.tile([128, 4, L], BF16, name="qfT")
        zT = consts.tile([128, HLOC, 8, 256], FP8, name="zT")
        kv_sb = consts.tile([128, 4, DH + 1], BF16, name="kv_sb")
        kv_bounce = consts.tile([64, 8, DH + 1], BF16, name="kv_bounce")
        ksb = consts.tile([128, 4, 2], BF16, name="ksb")
        sel2 = consts.tile([2, 128], mybir.dt.float32r, name="sel2")
        nw_sb = consts.tile([128, DL], F32, name="nw_sb")
        eps_sb = consts.tile([128, 1], F32, name="eps_sb")

        # ---------------- input DMAs ----------------
        # sync queue: x chunks (needed first); gpsimd: weights.
        nc.gpsimd.dma_start(
            w_inT_f8[:], w_inT_f8d.rearrange("(c p) d -> p c d", p=128)
        )
        xv = xT.rearrange("(c p) l -> p c l", p=128)
        for i in range(4):
            eng = nc.sync if i % 2 == 0 else nc.gpsimd
            eng.dma_start(x_sb[:, :, i * 1024 : (i + 1) * 1024], xv[:, :, i * 1024 : (i + 1) * 1024])
        nc.gpsimd.dma_start(w_qkvT[:], w_qkvTd.rearrange("(c p) e -> p c e", p=128))
        nc.sync.dma_start(
            nw_sb[:],
            norm_w.rearrange("(a d) -> a d", a=1).to_broadcast((128, DL)),
        )
        nc.gpsimd.dma_start(w_outT[:], w_outTd.rearrange("(c p) e -> p c e", p=128))
        nc.gpsimd.dma_start(w_inT_bf[:], w_inT_bfd.rearrange("(c p) d -> p c d", p=128))
        nc.sync.dma_start(xres[:], xresd.rearrange("(c p) l -> p c l", p=128))

        nc.vector.memset(eps_sb[:], 4.0 * EPS)
        # sel2[i, p] = 1 iff p//64 == i (partition-broadcast selector)
        nc.sync.dma_start(sel2[:], sel2d[:])

        # ---------------- phases 1-2: projections + kv state ----------------
        with (
            tc.tile_pool(name="w12", bufs=3) as w12,
            tc.tile_pool(name="ps_x", bufs=2, space="PSUM") as ps_x,
            tc.tile_pool(name="ps_kv", bufs=4, space="PSUM") as ps_kv,
            tc.tile_pool(name="ps_acc", bufs=1, space="PSUM") as ps_acc,
        ):
            # even heads accumulate in kv_ps_a, odd in kv_ps_b, across all
            # 32 token chunks (PSUM has_written semantics: one bank clear
            # at the start, then per-element accumulate/overwrite).
            kv_ps_a = ps_acc.tile([64, 4, DH + 1], F32, name="kv_ps_a")
            kv_ps_b = ps_acc.tile([64, 4, DH + 1], F32, name="kv_ps_b")

            for lt in range(8):  # 512-token tiles
                ls_l = lt * 512
                # x_proj -> xp (= 16*x_proj) in fp8, [dl, l] layout
                xp = w12.tile([128, 8, 512], FP8, name="xp")
                for dd in range(8):
                    ps = ps_x.tile([128, 512], F32, tag="mm", name="ps1")
                    for c2 in range(2):
                        nc.tensor.matmul(
                            ps[:],
                            w_inT_f8[:, 2 * c2 : 2 * c2 + 2, dd * 128 : (dd + 1) * 128],
                            x_sb[:, 2 * c2 : 2 * c2 + 2, ls_l : ls_l + 512],
                            start=(c2 == 0),
                            stop=(c2 == 1),
                            perf_mode=DR,
                        )
                    nc.any.tensor_copy(xp[:, dd, :], ps[:])

                # q-projection, transposed layout [dq, l]; elu+1 -> bf16
                for qq in range(4):
                    ps = ps_x.tile([128, 512], F32, tag="mm", name="psq")
                    for cc in range(4):
                        nc.tensor.matmul(
                            ps[:],
                            w_qkvT[:, 2 * cc : 2 * cc + 2, qq * 128 : (qq + 1) * 128],
                            xp[:, 2 * cc : 2 * cc + 2, :],
                            start=(cc == 0),
                            stop=(cc == 3),
                            perf_mode=DR,
                        )
                    eq = w12.tile([128, 512], BF16, name="eq")
                    rq = w12.tile([128, 512], BF16, name="rq")
                    nc.scalar.activation(eq[:], ps[:], ACTF.Exp, scale=1.0 / 256.0)
                    nc.vector.tensor_scalar(rq[:], ps[:], 0.0, 1.0 / 256.0, ALU.max, ALU.mult)
                    nc.vector.tensor_scalar(eq[:], eq[:], 1.0, None, ALU.min)
                    nc.vector.tensor_tensor(
                        qfT[:, qq, ls_l : ls_l + 512], eq[:], rq[:], ALU.add
                    )

                # k/v projection in [l, e] layout, 128-token subtiles
                for ls in range(4):
                    lhs = xp[:, :, ls * 128 : (ls + 1) * 128]
                    k_ps = ps_kv.tile([128, 512], F32, tag="kvp", name="k_ps")
                    v_ps = ps_kv.tile([128, 512], F32, tag="kvp", name="v_ps")
                    for cc in range(4):
                        nc.tensor.matmul(
                            k_ps[:],
                            lhs[:, 2 * cc : 2 * cc + 2, :],
                            w_qkvT[:, 2 * cc : 2 * cc + 2, 512:1024],
                            start=(cc == 0),
                            stop=(cc == 3),
                            perf_mode=DR,
                        )
                    for cc in range(4):
                        nc.tensor.matmul(
                            v_ps[:],
                            lhs[:, 2 * cc : 2 * cc + 2, :],
                            w_qkvT[:, 2 * cc : 2 * cc + 2, 1024:1536],
                            start=(cc == 0),
                            stop=(cc == 3),
                            perf_mode=DR,
                        )
                    kf = w12.tile([128, 512], BF16, name="kf")
                    ek = w12.tile([128, 512], BF16, name="ek")
                    nc.scalar.activation(ek[:], k_ps[:], ACTF.Exp, scale=1.0 / 256.0)
                    nc.vector.tensor_scalar(kf[:], k_ps[:], 0.0, 1.0 / 256.0, ALU.max, ALU.mult)
                    nc.vector.tensor_scalar(ek[:], ek[:], 1.0, None, ALU.min)
                    nc.vector.tensor_tensor(kf[:], kf[:], ek[:], ALU.add)

                    vt = w12.tile([128, HLOC, DH + 1], BF16, name="vt")
                    nc.vector.tensor_scalar(
                        vt[:, :, 0:DH],
                        v_ps[:].rearrange("p (h m) -> p h m", m=DH),
                        1.0 / 256.0,
                        None,
                        ALU.mult,
                    )
                    nc.vector.memset(vt[:, :, DH : DH + 1], 1.0)
                    first = lt == 0 and ls == 0
                    last = lt == 7 and ls == 3
                    for h in range(HLOC):
                        nc.tensor.matmul(
                            (kv_ps_a if h % 2 == 0 else kv_ps_b)[:, h // 2, :],
                            kf[:, h * DH : (h + 1) * DH],
                            vt[:, h, :],
                            start=(first and h < 2),
                            stop=(last and h >= 6),
                            skip_group_check=True,
                        )

            # kv state: cast to bf16, partition-move odd heads to 64..127
            nc.vector.tensor_copy(kv_bounce[:, 0:4, :], kv_ps_a[:])
            nc.vector.tensor_copy(kv_bounce[:, 4:8, :], kv_ps_b[:])
            nc.sync.dma_start(kv_sb[0:64, :, :], kv_bounce[:, 0:4, :])
            nc.sync.dma_start(kv_sb[64:128, :, :], kv_bounce[:, 4:8, :])

        # ---------------- normalizer: norm rows, recip, fold into qfT ----
        # ksb[64*i+d, s, i] = ksum of head 2s+i (kv col DH), else 0.
        nc.vector.memset(ksb[:], 0.0)
        nc.vector.tensor_copy(ksb[0:64, :, 0:1], kv_sb[0:64, :, DH : DH + 1])
        nc.vector.tensor_copy(ksb[64:128, :, 1:2], kv_sb[64:128, :, DH : DH + 1])

        # ---------------- phases 3-4 + normalizer, interleaved -------------
        # recip = 1/norm via fast-NR; the x16 z-scale is folded into sel2.
        with (
            tc.tile_pool(name="nsb", bufs=2) as nsb,
            tc.tile_pool(name="p34", bufs=3) as p34,
            tc.tile_pool(name="ps_n", bufs=1, space="PSUM") as ps_n,
            tc.tile_pool(name="psm", bufs=3, space="PSUM") as psm,
            tc.tile_pool(name="ps4", bufs=2, space="PSUM") as ps4p,
        ):
            pn = ps_n.tile([34, 512], F32, name="pn")

            def norm_slot(s):
                for c in range(8):
                    cs = slice(c * 512, (c + 1) * 512)
                    pb = (c % 2) * 32  # two pn slots in one bank, 32-aligned
                    nc.tensor.matmul(
                        pn[pb : pb + 2, :], ksb[:, s, :], qfT[:, s, cs],
                        start=True, stop=True, skip_group_check=True,
                    )
                    rc = nsb.tile([2, 512], F32, name="rc")
                    nc.vector.reciprocal_approx_fast(rc[:], pn[pb : pb + 2, :])
                    rcr = nsb.tile([2, 512], mybir.dt.float32r, name="rcr")
                    nc.gpsimd.tensor_copy(rcr[:], rc[:])
                    rb = psm.tile([128, 512], F32, tag="ps34", name="rb")
                    nc.tensor.matmul(rb[:], sel2[:], rcr[:], start=True, stop=True)
                    nc.vector.tensor_tensor(qfT[:, s, cs], qfT[:, s, cs], rb[:], ALU.mult)

            def phase3(h):
                s, par = h // 2, h % 2
                p0 = par * 64
                # zT[p, cc, m] = z[l=16m+2cc+(p//64), d=p%64] * 16
                for c2 in range(4):
                    ps3 = psm.tile([128, 2, 256], F32, tag="ps34", name="ps3")
                    for ci in range(2):
                        cc = 2 * c2 + ci
                        for pj in range(2):
                            nc.tensor.matmul(
                                ps3[pj * 64 : (pj + 1) * 64, ci, :],
                                kv_sb[p0 : p0 + 64, s, 0:DH],
                                qfT[p0 : p0 + 64, s, 2 * cc + pj : L : 16],
                                start=True,
                                stop=True,
                                skip_group_check=True,
                            )
                    nc.any.tensor_copy(zT[:, h, 2 * c2 : 2 * c2 + 2, :], ps3[:])

            def phase4(h):
                s, par = h // 2, h % 2
                for half in range(2):
                    lr = h * 256 + half * 128
                    ps4 = ps4p.tile([128, DL], F32, tag="p4", name="ps4")
                    for e in range(2):
                        es = slice(e * 512, (e + 1) * 512)
                        for t in range(4):
                            nc.tensor.matmul(
                                ps4[:, es],
                                zT[:, h, 2 * t : 2 * t + 2, half * 128 : (half + 1) * 128],
                                w_outT[:, 2 * t : 2 * t + 2, es],
                                start=(t == 0),
                                stop=False,
                                perf_mode=DR,
                            )
                        for cc in range(4):
                            nc.tensor.matmul(
                                ps4[:, es],
                                xres[:, cc, lr : lr + 128],
                                w_inT_bf[:, cc, es],
                                start=False,
                                stop=(cc == 3),
                            )
                    # RMS stats + final scale read PSUM directly
                    sq = p34.tile([128, DL], F32, name="sq")
                    ssum = p34.tile([128, 1], F32, name="ssum")
                    nc.scalar.activation(sq[:], ps4[:], ACTF.Square, accum_out=ssum[:])
                    # ps4 = 256*y, ssum = 2^16*sum(y^2); srt = 2*rms(y)
                    srt = p34.tile([128, 1], F32, name="srt")
                    nc.scalar.activation(
                        srt[:], ssum[:], ACTF.Sqrt, scale=1.0 / 16777216.0, bias=eps_sb[:]
                    )
                    rcp = p34.tile([128, 1], F32, name="rcp")
                    nc.vector.reciprocal_approx_fast(rcp[:], srt[:])
                    o = p34.tile([128, DL], F32, name="o")
                    nc.vector.tensor_scalar(o[:], ps4[:], rcp[:], 1.0 / 128.0, ALU.mult, ALU.mult)
                    nc.gpsimd.tensor_tensor(o[:], o[:], nw_sb[:], ALU.mult)
                    eng = nc.sync if (h + half) % 2 == 0 else nc.gpsimd
                    eng.dma_start(out[lr : lr + 128, :], o[:])

            # slot-s norms feed heads 2s/2s+1; stagger so PE stays fed while
            # the recip/scale chain of the next slot drains on DVE.
            norm_slot(0)
            norm_slot(1)
            phase3(0)
            phase3(1)
            phase4(0)
            norm_slot(2)
            phase3(2)
            phase4(1)
            phase3(3)
            phase4(2)
            norm_slot(3)
            phase3(4)
            phase4(3)
            phase3(5)
            phase4(4)
            phase3(6)
            phase4(5)
            phase3(7)
            phase4(6)
            phase4(7)


def build_program():
    if "nc" in _prog_cache:
        return _prog_cache["nc"]
    nc = bacc.Bacc(None, target_bir_lowering=False, debug=False)
    xT = nc.dram_tensor("xT", [CIN, L], FP8, kind="ExternalInput")
    xresd = nc.dram_tensor("xres", [CIN, LROWS], BF16, kind="ExternalInput")
    w_inT_f8d = nc.dram_tensor("w_inT_f8", [CIN, DL], FP8, kind="ExternalInput")
    w_inT_bfd = nc.dram_tensor("w_inT_bf", [CIN, DL], BF16, kind="ExternalInput")
    w_qkvTd = nc.dram_tensor("w_qkvT", [DL, ELOC], FP8, kind="ExternalInput")
    w_outTd = nc.dram_tensor("w_outT", [DL, DL], FP8, kind="ExternalInput")
    norm_w = nc.dram_tensor("norm_w", [DL], F32, kind="ExternalInput")
    sel2d = nc.dram_tensor("sel2", [2, 128], mybir.dt.float32r, kind="ExternalInput")
    out = nc.dram_tensor("out", [LROWS, DL], F32, kind="ExternalOutput")
    with tile.TileContext(nc) as tc:
        _build_body(
            tc, xT[:], xresd[:], w_inT_f8d[:], w_inT_bfd[:], w_qkvTd[:],
            w_outTd[:], norm_w[:], sel2d[:], out[:],
        )
    nc.compile()
    _prog_cache["nc"] = nc
    return nc


def make_in_maps(x, w_in, w_qkv, w_out, norm_w):
    import ml_dtypes

    bf16 = ml_dtypes.bfloat16
    f8e4 = mybir.dt.np(mybir.dt.float8e4)
    x = np.ascontiguousarray(np.asarray(x, dtype=np.float32))
    w_in = np.asarray(w_in, dtype=np.float32)
    w_qkv = np.asarray(w_qkv, dtype=np.float32)
    w_out = np.asarray(w_out, dtype=np.float32)
    norm_w = np.ascontiguousarray(np.asarray(norm_w, dtype=np.float32))

    w_inT_f8 = np.ascontiguousarray(w_in.T * W16).astype(f8e4)
    w_inT_bf = np.ascontiguousarray(w_in.T * (W16 * W16)).astype(bf16)
    # w_outT in (token-parity, d) partition layout, cc-chunked:
    # arr[p, cc, e] = w_out.T[(2cc + p//64)*64 + p%64, e] * 16
    wt = np.ascontiguousarray(w_out.T).reshape(8, 2, 64, DL)
    w_outT = np.ascontiguousarray(
        (wt.transpose(1, 2, 0, 3).reshape(128, 8, DL) * W16)
        .transpose(1, 0, 2)
        .reshape(DL, DL)
    ).astype(f8e4)

    sel2 = np.zeros((2, 128), dtype=np.float32)
    sel2[0, 0:64] = W16   # broadcast selector, carries the x16 z-scale
    sel2[1, 64:128] = W16

    in_maps = []
    for core in range(NCORES):
        b, g = core // 2, core % 2
        sl = slice(g * 512, (g + 1) * 512)
        wq = np.concatenate(
            [w_qkv[0:1024][sl], w_qkv[1024:2048][sl], w_qkv[2048:3072][sl]], axis=0
        )
        in_maps.append(
            {
                "xT": np.ascontiguousarray(x[b].T).astype(f8e4),
                "xres": np.ascontiguousarray(
                    x[b, g * LROWS : (g + 1) * LROWS].T
                ).astype(bf16),
                "w_inT_f8": w_inT_f8,
                "w_inT_bf": w_inT_bf,
                "w_qkvT": (np.ascontiguousarray(wq.T) * W16).astype(f8e4),
                "w_outT": w_outT,
                "norm_w": norm_w,
                "sel2": sel2,
            }
        )
    return in_maps


def run_on_cores(in_maps, trace=False):
    nc = build_program()
    return run_bass_kernel_spmd(nc, in_maps, list(range(NCORES)), trace=trace)


def assemble(results):
    out = np.empty((B, L, DL), np.float32)
    for core in range(NCORES):
        b, g = core // 2, core % 2
        out[b, g * LROWS : (g + 1) * LROWS] = results[core]["out"]
    return out


def kernel(x, w_in, w_qkv, w_out, norm_w):
    in_maps = make_in_maps(x, w_in, w_qkv, w_out, norm_w)
    res = run_on_cores(in_maps, trace=False)
    return assemble(res.results)


if __name__ == "__main__":
    nc = build_program()
    print("program built + compiled OK")


# revision 28
# speedup vs baseline: 1.5721x; 1.0927x over previous
"""Trainium2 Bass kernel for nn_AttentionOp_60988535603899.

Linear-attention (elu+1 feature map) block:
  x_proj = x @ w_in.T ; qkv = x_proj @ w_qkv.T ; per-head linear attention
  with kv-state; raw (B,H,L,D)->(B,L,H*D) reshape; out_proj; residual; RMS norm.

Sharding: 8 cores = 4 batches x 2 head-groups (8 heads each). No collectives.

v2 design (vs baseline):
  - All projection matmuls fp8 DoubleRow (x_proj, qkv, out_proj); residual
    recompute in bf16. Scales: w_in/w_qkv/w_out/z carry x16 each; the RMS
    normalization at the end cancels the combined scale automatically.
  - The attention output is produced directly TRANSPOSED (zT[(j%2)*64+d,
    cc, m] = z[l=16m+2cc+(j%2), d]) so out_proj needs no PE transposes at
    all (the transposes poisoned HAM clock-gating in the baseline) and z
    never round-trips through DRAM.
  - The 1/normalizer is folded into qfT before the z matmul: norm row per
    head via a block-diagonal ksum matmul, reciprocal, then a PE
    broadcast-matmul (sel outer product) to spread recip across the 128
    partitions of each qfT slot.
  - kv state accumulates directly in PSUM across all 32 token chunks
    (single has_written clear at the start) - no Vector adds, which were
    stalling the PE every l-tile in the baseline and re-throttling HAM.
"""

import sys

for _p in ("/opt/trn_rl_repo",):
    if _p not in sys.path:
        sys.path.insert(0, _p)

import numpy as np

import concourse.bass as bass  # noqa: F401  (bass must import before tile)
import concourse.mybir as mybir
import concourse.tile as tile
from concourse import bacc
from concourse.bass_utils import run_bass_kernel_spmd

F32 = mybir.dt.float32
BF16 = mybir.dt.bfloat16
FP8 = mybir.dt.float8e4
ALU = mybir.AluOpType
ACTF = mybir.ActivationFunctionType
DR = mybir.MatmulPerfMode.DoubleRow

B, L, CIN, DL = 4, 4096, 512, 1024
H, DH = 16, 64
HLOC = 8                  # heads per core
ELOC = 3 * HLOC * DH      # 1536 local qkv dims
LROWS = 2048              # output rows per core
EPS = float(np.finfo(np.float32).eps)
NCORES = 8
W16 = 16.0                # fp8 weight/activation scale

_prog_cache = {}


def _build_body(tc, xT, xresd, w_inT_f8d, w_inT_bfd, w_qkvTd, w_outTd, norm_w, sel2d, out):
    nc = tc.nc

    with tc.tile_pool(name="consts", bufs=1) as consts:
        # ---------------- persistent tiles ----------------
        x_sb = consts.tile([128, 4, L], FP8, name="x_sb")
        xres = consts.tile([128, 4, LROWS], BF16, name="xres")
        w_inT_f8 = consts.tile([128, 4, DL], FP8, name="w_inT_f8")
        w_inT_bf = consts.tile([128, 4, DL], BF16, name="w_inT_bf")
        w_qkvT = consts.tile([128, 8, ELOC], FP8, name="w_qkvT")
        w_outT = consts.tile([128, 8, DL], FP8, name="w_outT")
        qfT = consts.tile([128, 4, L], BF16, name="qfT")
        zT = consts.tile([128, HLOC, 8, 256], FP8, name="zT")
        kv_sb = consts.tile([128, 4, DH + 1], BF16, name="kv_sb")
        kv_bounce = consts.tile([64, 8, DH + 1], BF16, name="kv_bounce")
        ksb = consts.tile([128, 4, 2], BF16, name="ksb")
        sel2 = consts.tile([2, 128], mybir.dt.float32r, name="sel2")
        nw_sb = consts.tile([128, DL], F32, name="nw_sb")
        eps_sb = consts.tile([128, 1], F32, name="eps_sb")

        # ---------------- input DMAs ----------------
        # sync queue: x chunks (needed first); gpsimd: weights.
        nc.gpsimd.dma_start(
            w_inT_f8[:], w_inT_f8d.rearrange("(c p) d -> p c d", p=128)
        )
        xv = xT.rearrange("(c p) l -> p c l", p=128)
        for i in range(4):
            eng = nc.sync if i % 2 == 0 else nc.gpsimd
            eng.dma_start(x_sb[:, :, i * 1024 : (i + 1) * 1024], xv[:, :, i * 1024 : (i + 1) * 1024])
        nc.gpsimd.dma_start(w_qkvT[:], w_qkvTd.rearrange("(c p) e -> p c e", p=128))
        nc.sync.dma_start(
            nw_sb[:],
            norm_w.rearrange("(a d) -> a d", a=1).to_broadcast((128, DL)),
        )

        nc.vector.memset(eps_sb[:], 4.0 * EPS)
        # sel2[i, p] = 1 iff p//64 == i (partition-broadcast selector)
        nc.sync.dma_start(sel2[:], sel2d[:])

        # ---------------- phases 1-2: projections + kv state ----------------
        with (
            tc.tile_pool(name="w12", bufs=3) as w12,
            tc.tile_pool(name="ps_x", bufs=2, space="PSUM") as ps_x,
            tc.tile_pool(name="ps_kv", bufs=4, space="PSUM") as ps_kv,
            tc.tile_pool(name="ps_acc", bufs=1, space="PSUM") as ps_acc,
        ):
            # even heads accumulate in kv_ps_a, odd in kv_ps_b, across all
            # 32 token chunks (PSUM has_written semantics: one bank clear
            # at the start, then per-element accumulate/overwrite).
            kv_ps_a = ps_acc.tile([64, 4, DH + 1], F32, name="kv_ps_a")
            kv_ps_b = ps_acc.tile([64, 4, DH + 1], F32, name="kv_ps_b")

            xrv = xresd.rearrange("(c p) l -> p c l", p=128)
            for lt in range(8):  # 512-token tiles
                ls_l = lt * 512
                # deferred phase-3/4 loads: issued mid-loop so they don't
                # contend with the startup x/w_qkv DMA burst
                if lt == 2:
                    nc.gpsimd.dma_start(
                        w_outT[:], w_outTd.rearrange("(c p) e -> p c e", p=128)
                    )
                if lt == 3:
                    nc.gpsimd.dma_start(
                        w_inT_bf[:], w_inT_bfd.rearrange("(c p) d -> p c d", p=128)
                    )
                if lt >= 4:
                    q = lt - 4
                    nc.sync.dma_start(
                        xres[:, :, q * 512 : (q + 1) * 512],
                        xrv[:, :, q * 512 : (q + 1) * 512],
                    )
                # x_proj -> xp (= 16*x_proj) in fp8, [dl, l] layout
                xp = w12.tile([128, 8, 512], FP8, name="xp")
                for dd in range(8):
                    ps = ps_x.tile([128, 512], F32, tag="mm", name="ps1")
                    for c2 in range(2):
                        nc.tensor.matmul(
                            ps[:],
                            w_inT_f8[:, 2 * c2 : 2 * c2 + 2, dd * 128 : (dd + 1) * 128],
                            x_sb[:, 2 * c2 : 2 * c2 + 2, ls_l : ls_l + 512],
                            start=(c2 == 0),
                            stop=(c2 == 1),
                            perf_mode=DR,
                        )
                    nc.any.tensor_copy(xp[:, dd, :], ps[:])

                # q-projection, transposed layout [dq, l]; elu+1 -> bf16
                for qq in range(4):
                    ps = ps_x.tile([128, 512], F32, tag="mm", name="psq")
                    for cc in range(4):
                        nc.tensor.matmul(
                            ps[:],
                            w_qkvT[:, 2 * cc : 2 * cc + 2, qq * 128 : (qq + 1) * 128],
                            xp[:, 2 * cc : 2 * cc + 2, :],
                            start=(cc == 0),
                            stop=(cc == 3),
                            perf_mode=DR,
                        )
                    eq = w12.tile([128, 512], BF16, name="eq")
                    rq = w12.tile([128, 512], BF16, name="rq")
                    nc.scalar.activation(eq[:], ps[:], ACTF.Exp, scale=1.0 / 256.0)
                    nc.vector.tensor_scalar(rq[:], ps[:], 0.0, 1.0 / 256.0, ALU.max, ALU.mult)
                    nc.vector.tensor_scalar(eq[:], eq[:], 1.0, None, ALU.min)
                    nc.vector.tensor_tensor(
                        qfT[:, qq, ls_l : ls_l + 512], eq[:], rq[:], ALU.add
                    )

                # k/v projection in [l, e] layout, 128-token subtiles
                for ls in range(4):
                    lhs = xp[:, :, ls * 128 : (ls + 1) * 128]
                    k_ps = ps_kv.tile([128, 512], F32, tag="kvp", name="k_ps")
                    v_ps = ps_kv.tile([128, 512], F32, tag="kvp", name="v_ps")
                    for cc in range(4):
                        nc.tensor.matmul(
                            k_ps[:],
                            lhs[:, 2 * cc : 2 * cc + 2, :],
                            w_qkvT[:, 2 * cc : 2 * cc + 2, 512:1024],
                            start=(cc == 0),
                            stop=(cc == 3),
                            perf_mode=DR,
                        )
                    for cc in range(4):
                        nc.tensor.matmul(
                            v_ps[:],
                            lhs[:, 2 * cc : 2 * cc + 2, :],
                            w_qkvT[:, 2 * cc : 2 * cc + 2, 1024:1536],
                            start=(cc == 0),
                            stop=(cc == 3),
                            perf_mode=DR,
                        )
                    kf = w12.tile([128, 512], BF16, name="kf")
                    ek = w12.tile([128, 512], BF16, name="ek")
                    nc.scalar.activation(ek[:], k_ps[:], ACTF.Exp, scale=1.0 / 256.0)
                    nc.vector.tensor_scalar(kf[:], k_ps[:], 0.0, 1.0 / 256.0, ALU.max, ALU.mult)
                    nc.vector.tensor_scalar(ek[:], ek[:], 1.0, None, ALU.min)
                    nc.vector.tensor_tensor(kf[:], kf[:], ek[:], ALU.add)

                    vt = w12.tile([128, HLOC, DH + 1], BF16, name="vt")
                    nc.vector.tensor_scalar(
                        vt[:, :, 0:DH],
                        v_ps[:].rearrange("p (h m) -> p h m", m=DH),
                        1.0 / 256.0,
                        None,
                        ALU.mult,
                    )
                    nc.vector.memset(vt[:, :, DH : DH + 1], 1.0)
                    first = lt == 0 and ls == 0
                    last = lt == 7 and ls == 3
                    for h in range(HLOC):
                        nc.tensor.matmul(
                            (kv_ps_a if h % 2 == 0 else kv_ps_b)[:, h // 2, :],
                            kf[:, h * DH : (h + 1) * DH],
                            vt[:, h, :],
                            start=(first and h < 2),
                            stop=(last and h >= 6),
                            skip_group_check=True,
                        )

            # kv state: cast to bf16, partition-move odd heads to 64..127
            nc.vector.tensor_copy(kv_bounce[:, 0:4, :], kv_ps_a[:])
            nc.vector.tensor_copy(kv_bounce[:, 4:8, :], kv_ps_b[:])
            nc.sync.dma_start(kv_sb[0:64, :, :], kv_bounce[:, 0:4, :])
            nc.sync.dma_start(kv_sb[64:128, :, :], kv_bounce[:, 4:8, :])

        # ---------------- normalizer: norm rows, recip, fold into qfT ----
        # ksb[64*i+d, s, i] = ksum of head 2s+i (kv col DH), else 0.
        nc.vector.memset(ksb[:], 0.0)
        nc.vector.tensor_copy(ksb[0:64, :, 0:1], kv_sb[0:64, :, DH : DH + 1])
        nc.vector.tensor_copy(ksb[64:128, :, 1:2], kv_sb[64:128, :, DH : DH + 1])

        # ---------------- phases 3-4 + normalizer, interleaved -------------
        # recip = 1/norm via fast-NR; the x16 z-scale is folded into sel2.
        with (
            tc.tile_pool(name="nsb", bufs=2) as nsb,
            tc.tile_pool(name="p34", bufs=3) as p34,
            tc.tile_pool(name="ps_n", bufs=1, space="PSUM") as ps_n,
            tc.tile_pool(name="psm", bufs=3, space="PSUM") as psm,
            tc.tile_pool(name="ps4", bufs=2, space="PSUM") as ps4p,
        ):
            pn = ps_n.tile([34, 512], F32, name="pn")

            def norm_slot(s):
                for c in range(8):
                    cs = slice(c * 512, (c + 1) * 512)
                    pb = (c % 2) * 32  # two pn slots in one bank, 32-aligned
                    nc.tensor.matmul(
                        pn[pb : pb + 2, :], ksb[:, s, :], qfT[:, s, cs],
                        start=True, stop=True, skip_group_check=True,
                    )
                    rc = nsb.tile([2, 512], F32, name="rc")
                    nc.vector.reciprocal_approx_fast(rc[:], pn[pb : pb + 2, :])
                    rcr = nsb.tile([2, 512], mybir.dt.float32r, name="rcr")
                    nc.scalar.activation(rcr[:], rc[:], ACTF.Copy)
                    rb = psm.tile([128, 512], F32, tag="ps34", name="rb")
                    nc.tensor.matmul(rb[:], sel2[:], rcr[:], start=True, stop=True)
                    nc.vector.tensor_tensor(qfT[:, s, cs], qfT[:, s, cs], rb[:], ALU.mult)

            def phase3(h):
                s, par = h // 2, h % 2
                p0 = par * 64
                # zT[p, cc, m] = z[l=16m+2cc+(p//64), d=p%64] * 16
                for c2 in range(4):
                    ps3 = psm.tile([128, 2, 256], F32, tag="ps34", name="ps3")
                    for ci in range(2):
                        cc = 2 * c2 + ci
                        for pj in range(2):
                            nc.tensor.matmul(
                                ps3[pj * 64 : (pj + 1) * 64, ci, :],
                                kv_sb[p0 : p0 + 64, s, 0:DH],
                                qfT[p0 : p0 + 64, s, 2 * cc + pj : L : 16],
                                start=True,
                                stop=True,
                                skip_group_check=True,
                            )
                    nc.any.tensor_copy(zT[:, h, 2 * c2 : 2 * c2 + 2, :], ps3[:])

            def phase4(h):
                s, par = h // 2, h % 2
                for half in range(2):
                    lr = h * 256 + half * 128
                    ps4 = ps4p.tile([128, DL], F32, tag="p4", name="ps4")
                    for e in range(2):
                        es = slice(e * 512, (e + 1) * 512)
                        for t in range(4):
                            nc.tensor.matmul(
                                ps4[:, es],
                                zT[:, h, 2 * t : 2 * t + 2, half * 128 : (half + 1) * 128],
                                w_outT[:, 2 * t : 2 * t + 2, es],
                                start=(t == 0),
                                stop=False,
                                perf_mode=DR,
                            )
                        for cc in range(4):
                            nc.tensor.matmul(
                                ps4[:, es],
                                xres[:, cc, lr : lr + 128],
                                w_inT_bf[:, cc, es],
                                start=False,
                                stop=(cc == 3),
                            )
                    # RMS stats + final scale read PSUM directly
                    sq = p34.tile([128, DL], F32, name="sq")
                    ssum = p34.tile([128, 1], F32, name="ssum")
                    nc.scalar.activation(sq[:], ps4[:], ACTF.Square, accum_out=ssum[:])
                    # ps4 = 256*y, ssum = 2^16*sum(y^2); srt = 2*rms(y)
                    srt = p34.tile([128, 1], F32, name="srt")
                    nc.scalar.activation(
                        srt[:], ssum[:], ACTF.Sqrt, scale=1.0 / 16777216.0, bias=eps_sb[:]
                    )
                    rcp = p34.tile([128, 1], F32, name="rcp")
                    nc.vector.reciprocal_approx_fast(rcp[:], srt[:])
                    o = p34.tile([128, DL], F32, name="o")
                    nc.vector.tensor_scalar(o[:], ps4[:], rcp[:], 1.0 / 128.0, ALU.mult, ALU.mult)
                    nc.gpsimd.tensor_tensor(o[:], o[:], nw_sb[:], ALU.mult)
                    eng = nc.sync if (h + half) % 2 == 0 else nc.gpsimd
                    eng.dma_start(out[lr : lr + 128, :], o[:])

            # slot-s norms feed heads 2s/2s+1; stagger so PE stays fed while
            # the recip/scale chain of the next slot drains on DVE.
            norm_slot(0)
            norm_slot(1)
            phase3(0)
            phase3(1)
            phase4(0)
            norm_slot(2)
            phase3(2)
            phase4(1)
            phase3(3)
            phase4(2)
            norm_slot(3)
            phase3(4)
            phase4(3)
            phase3(5)
            phase4(4)
            phase3(6)
            phase4(5)
            phase3(7)
            phase4(6)
            phase4(7)


def build_program():
    if "nc" in _prog_cache:
        return _prog_cache["nc"]
    nc = bacc.Bacc(None, target_bir_lowering=False, debug=False)
    xT = nc.dram_tensor("xT", [CIN, L], FP8, kind="ExternalInput")
    xresd = nc.dram_tensor("xres", [CIN, LROWS], BF16, kind="ExternalInput")
    w_inT_f8d = nc.dram_tensor("w_inT_f8", [CIN, DL], FP8, kind="ExternalInput")
    w_inT_bfd = nc.dram_tensor("w_inT_bf", [CIN, DL], BF16, kind="ExternalInput")
    w_qkvTd = nc.dram_tensor("w_qkvT", [DL, ELOC], FP8, kind="ExternalInput")
    w_outTd = nc.dram_tensor("w_outT", [DL, DL], FP8, kind="ExternalInput")
    norm_w = nc.dram_tensor("norm_w", [DL], F32, kind="ExternalInput")
    sel2d = nc.dram_tensor("sel2", [2, 128], mybir.dt.float32r, kind="ExternalInput")
    out = nc.dram_tensor("out", [LROWS, DL], F32, kind="ExternalOutput")
    with tile.TileContext(nc) as tc:
        _build_body(
            tc, xT[:], xresd[:], w_inT_f8d[:], w_inT_bfd[:], w_qkvTd[:],
            w_outTd[:], norm_w[:], sel2d[:], out[:],
        )
    nc.compile()
    _prog_cache["nc"] = nc
    return nc


def make_in_maps(x, w_in, w_qkv, w_out, norm_w):
    import ml_dtypes

    bf16 = ml_dtypes.bfloat16
    f8e4 = mybir.dt.np(mybir.dt.float8e4)
    x = np.ascontiguousarray(np.asarray(x, dtype=np.float32))
    w_in = np.asarray(w_in, dtype=np.float32)
    w_qkv = np.asarray(w_qkv, dtype=np.float32)
    w_out = np.asarray(w_out, dtype=np.float32)
    norm_w = np.ascontiguousarray(np.asarray(norm_w, dtype=np.float32))

    w_inT_f8 = np.ascontiguousarray(w_in.T * W16).astype(f8e4)
    w_inT_bf = np.ascontiguousarray(w_in.T * (W16 * W16)).astype(bf16)
    # w_outT in (token-parity, d) partition layout, cc-chunked:
    # arr[p, cc, e] = w_out.T[(2cc + p//64)*64 + p%64, e] * 16
    wt = np.ascontiguousarray(w_out.T).reshape(8, 2, 64, DL)
    w_outT = np.ascontiguousarray(
        (wt.transpose(1, 2, 0, 3).reshape(128, 8, DL) * W16)
        .transpose(1, 0, 2)
        .reshape(DL, DL)
    ).astype(f8e4)

    sel2 = np.zeros((2, 128), dtype=np.float32)
    sel2[0, 0:64] = W16   # broadcast selector, carries the x16 z-scale
    sel2[1, 64:128] = W16

    in_maps = []
    for core in range(NCORES):
        b, g = core // 2, core % 2
        sl = slice(g * 512, (g + 1) * 512)
        wq = np.concatenate(
            [w_qkv[0:1024][sl], w_qkv[1024:2048][sl], w_qkv[2048:3072][sl]], axis=0
        )
        in_maps.append(
            {
                "xT": np.ascontiguousarray(x[b].T).astype(f8e4),
                "xres": np.ascontiguousarray(
                    x[b, g * LROWS : (g + 1) * LROWS].T
                ).astype(bf16),
                "w_inT_f8": w_inT_f8,
                "w_inT_bf": w_inT_bf,
                "w_qkvT": (np.ascontiguousarray(wq.T) * W16).astype(f8e4),
                "w_outT": w_outT,
                "norm_w": norm_w,
                "sel2": sel2,
            }
        )
    return in_maps


def run_on_cores(in_maps, trace=False):
    nc = build_program()
    return run_bass_kernel_spmd(nc, in_maps, list(range(NCORES)), trace=trace)


def assemble(results):
    out = np.empty((B, L, DL), np.float32)
    for core in range(NCORES):
        b, g = core // 2, core % 2
        out[b, g * LROWS : (g + 1) * LROWS] = results[core]["out"]
    return out


def kernel(x, w_in, w_qkv, w_out, norm_w):
    in_maps = make_in_maps(x, w_in, w_qkv, w_out, norm_w)
    res = run_on_cores(in_maps, trace=False)
    return assemble(res.results)


if __name__ == "__main__":
    nc = build_program()
    print("program built + compiled OK")


# revision 32
# speedup vs baseline: 1.6003x; 1.0179x over previous
"""Trainium2 Bass kernel for nn_AttentionOp_60988535603899.

Linear-attention (elu+1 feature map) block:
  x_proj = x @ w_in.T ; qkv = x_proj @ w_qkv.T ; per-head linear attention
  with kv-state; raw (B,H,L,D)->(B,L,H*D) reshape; out_proj; residual; RMS norm.

Sharding: 8 cores = 4 batches x 2 head-groups (8 heads each). No collectives.

v2 design (vs baseline):
  - All projection matmuls fp8 DoubleRow (x_proj, qkv, out_proj); residual
    recompute in bf16. Scales: w_in/w_qkv/w_out/z carry x16 each; the RMS
    normalization at the end cancels the combined scale automatically.
  - The attention output is produced directly TRANSPOSED (zT[(j%2)*64+d,
    cc, m] = z[l=16m+2cc+(j%2), d]) so out_proj needs no PE transposes at
    all (the transposes poisoned HAM clock-gating in the baseline) and z
    never round-trips through DRAM.
  - The 1/normalizer is folded into qfT before the z matmul: norm row per
    head via a block-diagonal ksum matmul, reciprocal, then a PE
    broadcast-matmul (sel outer product) to spread recip across the 128
    partitions of each qfT slot.
  - kv state accumulates directly in PSUM across all 32 token chunks
    (single has_written clear at the start) - no Vector adds, which were
    stalling the PE every l-tile in the baseline and re-throttling HAM.
"""

import sys

for _p in ("/opt/trn_rl_repo",):
    if _p not in sys.path:
        sys.path.insert(0, _p)

import numpy as np

import concourse.bass as bass  # noqa: F401  (bass must import before tile)
import concourse.mybir as mybir
import concourse.tile as tile
from concourse import bacc
from concourse.bass_utils import run_bass_kernel_spmd

F32 = mybir.dt.float32
BF16 = mybir.dt.bfloat16
FP8 = mybir.dt.float8e4
ALU = mybir.AluOpType
ACTF = mybir.ActivationFunctionType
DR = mybir.MatmulPerfMode.DoubleRow

B, L, CIN, DL = 4, 4096, 512, 1024
H, DH = 16, 64
HLOC = 8                  # heads per core
ELOC = 3 * HLOC * DH      # 1536 local qkv dims
LROWS = 2048              # output rows per core
EPS = float(np.finfo(np.float32).eps)
NCORES = 8
W16 = 16.0                # fp8 weight/activation scale

_prog_cache = {}


def _build_body(tc, xT, xresd, w_inT_f8d, w_inT_bfd, w_qkvTd, w_outTd, norm_w, sel2d, out):
    nc = tc.nc

    with tc.tile_pool(name="consts", bufs=1) as consts:
        # ---------------- persistent tiles ----------------
        x_sb = consts.tile([128, 4, L], FP8, name="x_sb")
        xres = consts.tile([128, 4, LROWS], BF16, name="xres")
        w_inT_f8 = consts.tile([128, 4, DL], FP8, name="w_inT_f8")
        w_inT_bf = consts.tile([128, 4, DL], BF16, name="w_inT_bf")
        w_qkvT = consts.tile([128, 8, ELOC], FP8, name="w_qkvT")
        w_outT = consts.tile([128, 8, DL], FP8, name="w_outT")
        qfT = consts.tile([128, 4, L], BF16, name="qfT")
        zT = consts.tile([128, HLOC, 8, 256], FP8, name="zT")
        kv_sb = consts.tile([128, 4, DH + 1], BF16, name="kv_sb")
        kv_bounce = consts.tile([64, 8, DH + 1], BF16, name="kv_bounce")
        ksb = consts.tile([128, 4, 2], BF16, name="ksb")
        sel2 = consts.tile([2, 128], mybir.dt.float32r, name="sel2")
        nw_sb = consts.tile([128, DL], F32, name="nw_sb")
        eps_sb = consts.tile([128, 1], F32, name="eps_sb")

        # ---------------- input DMAs ----------------
        # sync queue: x chunks (needed first); gpsimd: weights.
        nc.gpsimd.dma_start(
            w_inT_f8[:], w_inT_f8d.rearrange("(c p) d -> p c d", p=128)
        )
        xv = xT.rearrange("(c p) l -> p c l", p=128)
        for i in range(2):
            eng = nc.sync if i % 2 == 0 else nc.gpsimd
            eng.dma_start(x_sb[:, :, i * 1024 : (i + 1) * 1024], xv[:, :, i * 1024 : (i + 1) * 1024])
        nc.gpsimd.dma_start(w_qkvT[:], w_qkvTd.rearrange("(c p) e -> p c e", p=128))

        nc.vector.memset(eps_sb[:], 4.0 * EPS)
        # sel2[i, p] = 1 iff p//64 == i (partition-broadcast selector)
        nc.sync.dma_start(sel2[:], sel2d[:])

        # ---------------- phases 1-2: projections + kv state ----------------
        with (
            tc.tile_pool(name="w12", bufs=3) as w12,
            tc.tile_pool(name="ps_x", bufs=2, space="PSUM") as ps_x,
            tc.tile_pool(name="ps_kv", bufs=4, space="PSUM") as ps_kv,
            tc.tile_pool(name="ps_acc", bufs=1, space="PSUM") as ps_acc,
        ):
            # even heads accumulate in kv_ps_a, odd in kv_ps_b, across all
            # 32 token chunks (PSUM has_written semantics: one bank clear
            # at the start, then per-element accumulate/overwrite).
            kv_ps_a = ps_acc.tile([64, 4, DH + 1], F32, name="kv_ps_a")
            kv_ps_b = ps_acc.tile([64, 4, DH + 1], F32, name="kv_ps_b")

            xrv = xresd.rearrange("(c p) l -> p c l", p=128)
            for lt in range(8):  # 512-token tiles
                ls_l = lt * 512
                # deferred phase-3/4 loads: issued mid-loop so they don't
                # contend with the startup x/w_qkv DMA burst
                if lt == 1:
                    nc.sync.dma_start(x_sb[:, :, 2048:3072], xv[:, :, 2048:3072])
                if lt == 2:
                    nc.gpsimd.dma_start(x_sb[:, :, 3072:4096], xv[:, :, 3072:4096])
                    nc.gpsimd.dma_start(
                        w_outT[:], w_outTd.rearrange("(c p) e -> p c e", p=128)
                    )
                if lt == 3:
                    nc.gpsimd.dma_start(
                        w_inT_bf[:], w_inT_bfd.rearrange("(c p) d -> p c d", p=128)
                    )
                if lt == 5:
                    nc.sync.dma_start(
                        nw_sb[:],
                        norm_w.rearrange("(a d) -> a d", a=1).to_broadcast((128, DL)),
                    )
                if lt >= 4:
                    q = lt - 4
                    nc.sync.dma_start(
                        xres[:, :, q * 512 : (q + 1) * 512],
                        xrv[:, :, q * 512 : (q + 1) * 512],
                    )
                # x_proj -> xp (= 16*x_proj) in fp8, [dl, l] layout
                xp = w12.tile([128, 8, 512], FP8, name="xp")
                for dd in range(8):
                    ps = ps_x.tile([128, 512], F32, tag="mm", name="ps1")
                    for c2 in range(2):
                        nc.tensor.matmul(
                            ps[:],
                            w_inT_f8[:, 2 * c2 : 2 * c2 + 2, dd * 128 : (dd + 1) * 128],
                            x_sb[:, 2 * c2 : 2 * c2 + 2, ls_l : ls_l + 512],
                            start=(c2 == 0),
                            stop=(c2 == 1),
                            perf_mode=DR,
                        )
                    nc.any.tensor_copy(xp[:, dd, :], ps[:])

                # q-projection, transposed layout [dq, l]; elu+1 -> bf16
                for qq in range(4):
                    ps = ps_x.tile([128, 512], F32, tag="mm", name="psq")
                    for cc in range(4):
                        nc.tensor.matmul(
                            ps[:],
                            w_qkvT[:, 2 * cc : 2 * cc + 2, qq * 128 : (qq + 1) * 128],
                            xp[:, 2 * cc : 2 * cc + 2, :],
                            start=(cc == 0),
                            stop=(cc == 3),
                            perf_mode=DR,
                        )
                    eq = w12.tile([128, 512], BF16, name="eq")
                    rq = w12.tile([128, 512], BF16, name="rq")
                    nc.scalar.activation(eq[:], ps[:], ACTF.Exp, scale=1.0 / 256.0)
                    nc.vector.tensor_scalar(rq[:], ps[:], 0.0, 1.0 / 256.0, ALU.max, ALU.mult)
                    nc.vector.tensor_scalar(eq[:], eq[:], 1.0, None, ALU.min)
                    nc.vector.tensor_tensor(
                        qfT[:, qq, ls_l : ls_l + 512], eq[:], rq[:], ALU.add
                    )

                # k/v projection in [l, e] layout, 128-token subtiles
                for ls in range(4):
                    lhs = xp[:, :, ls * 128 : (ls + 1) * 128]
                    k_ps = ps_kv.tile([128, 512], F32, tag="kvp", name="k_ps")
                    v_ps = ps_kv.tile([128, 512], F32, tag="kvp", name="v_ps")
                    for cc in range(4):
                        nc.tensor.matmul(
                            k_ps[:],
                            lhs[:, 2 * cc : 2 * cc + 2, :],
                            w_qkvT[:, 2 * cc : 2 * cc + 2, 512:1024],
                            start=(cc == 0),
                            stop=(cc == 3),
                            perf_mode=DR,
                        )
                    for cc in range(4):
                        nc.tensor.matmul(
                            v_ps[:],
                            lhs[:, 2 * cc : 2 * cc + 2, :],
                            w_qkvT[:, 2 * cc : 2 * cc + 2, 1024:1536],
                            start=(cc == 0),
                            stop=(cc == 3),
                            perf_mode=DR,
                        )
                    kf = w12.tile([128, 512], BF16, name="kf")
                    ek = w12.tile([128, 512], BF16, name="ek")
                    nc.scalar.activation(ek[:], k_ps[:], ACTF.Exp, scale=1.0 / 256.0)
                    nc.vector.tensor_scalar(kf[:], k_ps[:], 0.0, 1.0 / 256.0, ALU.max, ALU.mult)
                    nc.vector.tensor_scalar(ek[:], ek[:], 1.0, None, ALU.min)
                    nc.vector.tensor_tensor(kf[:], kf[:], ek[:], ALU.add)

                    vt = w12.tile([128, HLOC, DH + 1], BF16, name="vt")
                    nc.vector.tensor_scalar(
                        vt[:, :, 0:DH],
                        v_ps[:].rearrange("p (h m) -> p h m", m=DH),
                        1.0 / 256.0,
                        None,
                        ALU.mult,
                    )
                    nc.vector.memset(vt[:, :, DH : DH + 1], 1.0)
                    first = lt == 0 and ls == 0
                    last = lt == 7 and ls == 3
                    for h in range(HLOC):
                        nc.tensor.matmul(
                            (kv_ps_a if h % 2 == 0 else kv_ps_b)[:, h // 2, :],
                            kf[:, h * DH : (h + 1) * DH],
                            vt[:, h, :],
                            start=(first and h < 2),
                            stop=(last and h >= 6),
                            skip_group_check=True,
                        )

            # kv state: cast to bf16, partition-move odd heads to 64..127
            nc.vector.tensor_copy(kv_bounce[:, 0:4, :], kv_ps_a[:])
            nc.vector.tensor_copy(kv_bounce[:, 4:8, :], kv_ps_b[:])
            nc.sync.dma_start(kv_sb[0:64, :, :], kv_bounce[:, 0:4, :])
            nc.sync.dma_start(kv_sb[64:128, :, :], kv_bounce[:, 4:8, :])

        # ---------------- normalizer: norm rows, recip, fold into qfT ----
        # ksb[64*i+d, s, i] = ksum of head 2s+i (kv col DH), else 0.
        nc.vector.memset(ksb[:], 0.0)
        nc.vector.tensor_copy(ksb[0:64, :, 0:1], kv_sb[0:64, :, DH : DH + 1])
        nc.vector.tensor_copy(ksb[64:128, :, 1:2], kv_sb[64:128, :, DH : DH + 1])

        # ---------------- phases 3-4 + normalizer, interleaved -------------
        # recip = 1/norm via fast-NR; the x16 z-scale is folded into sel2.
        with (
            tc.tile_pool(name="nsb", bufs=2) as nsb,
            tc.tile_pool(name="p34", bufs=3) as p34,
            tc.tile_pool(name="ps_n", bufs=1, space="PSUM") as ps_n,
            tc.tile_pool(name="psm", bufs=3, space="PSUM") as psm,
            tc.tile_pool(name="ps4", bufs=2, space="PSUM") as ps4p,
        ):
            pn = ps_n.tile([34, 512], F32, name="pn")

            def norm_slot(s):
                for c in range(8):
                    cs = slice(c * 512, (c + 1) * 512)
                    pb = (c % 2) * 32  # two pn slots in one bank, 32-aligned
                    nc.tensor.matmul(
                        pn[pb : pb + 2, :], ksb[:, s, :], qfT[:, s, cs],
                        start=True, stop=True, skip_group_check=True,
                    )
                    rc = nsb.tile([2, 512], F32, name="rc")
                    nc.vector.reciprocal_approx_fast(rc[:], pn[pb : pb + 2, :])
                    rcr = nsb.tile([2, 512], mybir.dt.float32r, name="rcr")
                    nc.scalar.activation(rcr[:], rc[:], ACTF.Copy)
                    rb = psm.tile([128, 512], F32, tag="ps34", name="rb")
                    nc.tensor.matmul(rb[:], sel2[:], rcr[:], start=True, stop=True)
                    nc.vector.tensor_tensor(qfT[:, s, cs], qfT[:, s, cs], rb[:], ALU.mult)

            def phase3(h):
                s, par = h // 2, h % 2
                p0 = par * 64
                # zT[p, cc, m] = z[l=16m+2cc+(p//64), d=p%64] * 16
                for c2 in range(4):
                    ps3 = psm.tile([128, 2, 256], F32, tag="ps34", name="ps3")
                    for ci in range(2):
                        cc = 2 * c2 + ci
                        for pj in range(2):
                            nc.tensor.matmul(
                                ps3[pj * 64 : (pj + 1) * 64, ci, :],
                                kv_sb[p0 : p0 + 64, s, 0:DH],
                                qfT[p0 : p0 + 64, s, 2 * cc + pj : L : 16],
                                start=True,
                                stop=True,
                                skip_group_check=True,
                            )
                    nc.any.tensor_copy(zT[:, h, 2 * c2 : 2 * c2 + 2, :], ps3[:])

            def phase4(h):
                s, par = h // 2, h % 2
                for half in range(2):
                    lr = h * 256 + half * 128
                    ps4 = ps4p.tile([128, DL], F32, tag="p4", name="ps4")
                    for e in range(2):
                        es = slice(e * 512, (e + 1) * 512)
                        for t in range(4):
                            nc.tensor.matmul(
                                ps4[:, es],
                                zT[:, h, 2 * t : 2 * t + 2, half * 128 : (half + 1) * 128],
                                w_outT[:, 2 * t : 2 * t + 2, es],
                                start=(t == 0),
                                stop=False,
                                perf_mode=DR,
                            )
                        for cc in range(4):
                            nc.tensor.matmul(
                                ps4[:, es],
                                xres[:, cc, lr : lr + 128],
                                w_inT_bf[:, cc, es],
                                start=False,
                                stop=(cc == 3),
                            )
                    # RMS stats + final scale read PSUM directly
                    sq = p34.tile([128, DL], F32, name="sq")
                    ssum = p34.tile([128, 1], F32, name="ssum")
                    nc.scalar.activation(sq[:], ps4[:], ACTF.Square, accum_out=ssum[:])
                    # ps4 = 256*y, ssum = 2^16*sum(y^2); srt = 2*rms(y)
                    srt = p34.tile([128, 1], F32, name="srt")
                    nc.scalar.activation(
                        srt[:], ssum[:], ACTF.Sqrt, scale=1.0 / 16777216.0, bias=eps_sb[:]
                    )
                    rcp = p34.tile([128, 1], F32, name="rcp")
                    nc.vector.reciprocal_approx_fast(rcp[:], srt[:])
                    # o = (ps4 * rcp) * nw_sb  (norm_w pre-scaled by 1/128 on host)
                    o = p34.tile([128, DL], F32, name="o")
                    nc.vector.scalar_tensor_tensor(
                        o[:], ps4[:], rcp[:], nw_sb[:], ALU.mult, ALU.mult
                    )
                    eng = nc.sync if (h + half) % 2 == 0 else nc.gpsimd
                    eng.dma_start(out[lr : lr + 128, :], o[:])

            # slot-s norms feed heads 2s/2s+1; stagger so PE stays fed while
            # the recip/scale chain of the next slot drains on DVE.
            norm_slot(0)
            norm_slot(1)
            phase3(0)
            phase3(1)
            phase4(0)
            norm_slot(2)
            phase3(2)
            phase4(1)
            phase3(3)
            phase4(2)
            norm_slot(3)
            phase3(4)
            phase4(3)
            phase3(5)
            phase4(4)
            phase3(6)
            phase4(5)
            phase3(7)
            phase4(6)
            phase4(7)


def build_program():
    if "nc" in _prog_cache:
        return _prog_cache["nc"]
    nc = bacc.Bacc(None, target_bir_lowering=False, debug=False)
    xT = nc.dram_tensor("xT", [CIN, L], FP8, kind="ExternalInput")
    xresd = nc.dram_tensor("xres", [CIN, LROWS], BF16, kind="ExternalInput")
    w_inT_f8d = nc.dram_tensor("w_inT_f8", [CIN, DL], FP8, kind="ExternalInput")
    w_inT_bfd = nc.dram_tensor("w_inT_bf", [CIN, DL], BF16, kind="ExternalInput")
    w_qkvTd = nc.dram_tensor("w_qkvT", [DL, ELOC], FP8, kind="ExternalInput")
    w_outTd = nc.dram_tensor("w_outT", [DL, DL], FP8, kind="ExternalInput")
    norm_w = nc.dram_tensor("norm_w", [DL], F32, kind="ExternalInput")
    sel2d = nc.dram_tensor("sel2", [2, 128], mybir.dt.float32r, kind="ExternalInput")
    out = nc.dram_tensor("out", [LROWS, DL], F32, kind="ExternalOutput")
    with tile.TileContext(nc) as tc:
        _build_body(
            tc, xT[:], xresd[:], w_inT_f8d[:], w_inT_bfd[:], w_qkvTd[:],
            w_outTd[:], norm_w[:], sel2d[:], out[:],
        )
    nc.compile()
    _prog_cache["nc"] = nc
    return nc


def make_in_maps(x, w_in, w_qkv, w_out, norm_w):
    import ml_dtypes

    bf16 = ml_dtypes.bfloat16
    f8e4 = mybir.dt.np(mybir.dt.float8e4)
    x = np.ascontiguousarray(np.asarray(x, dtype=np.float32))
    w_in = np.asarray(w_in, dtype=np.float32)
    w_qkv = np.asarray(w_qkv, dtype=np.float32)
    w_out = np.asarray(w_out, dtype=np.float32)
    norm_w = np.ascontiguousarray(np.asarray(norm_w, dtype=np.float32))

    w_inT_f8 = np.ascontiguousarray(w_in.T * W16).astype(f8e4)
    w_inT_bf = np.ascontiguousarray(w_in.T * (W16 * W16)).astype(bf16)
    # w_outT in (token-parity, d) partition layout, cc-chunked:
    # arr[p, cc, e] = w_out.T[(2cc + p//64)*64 + p%64, e] * 16
    wt = np.ascontiguousarray(w_out.T).reshape(8, 2, 64, DL)
    w_outT = np.ascontiguousarray(
        (wt.transpose(1, 2, 0, 3).reshape(128, 8, DL) * W16)
        .transpose(1, 0, 2)
        .reshape(DL, DL)
    ).astype(f8e4)

    sel2 = np.zeros((2, 128), dtype=np.float32)
    sel2[0, 0:64] = W16   # broadcast selector, carries the x16 z-scale
    sel2[1, 64:128] = W16

    in_maps = []
    for core in range(NCORES):
        b, g = core // 2, core % 2
        sl = slice(g * 512, (g + 1) * 512)
        wq = np.concatenate(
            [w_qkv[0:1024][sl], w_qkv[1024:2048][sl], w_qkv[2048:3072][sl]], axis=0
        )
        in_maps.append(
            {
                "xT": np.ascontiguousarray(x[b].T).astype(f8e4),
                "xres": np.ascontiguousarray(
                    x[b, g * LROWS : (g + 1) * LROWS].T
                ).astype(bf16),
                "w_inT_f8": w_inT_f8,
                "w_inT_bf": w_inT_bf,
                "w_qkvT": (np.ascontiguousarray(wq.T) * W16).astype(f8e4),
                "w_outT": w_outT,
                "norm_w": norm_w / 128.0,
                "sel2": sel2,
            }
        )
    return in_maps


def run_on_cores(in_maps, trace=False):
    nc = build_program()
    return run_bass_kernel_spmd(nc, in_maps, list(range(NCORES)), trace=trace)


def assemble(results):
    out = np.empty((B, L, DL), np.float32)
    for core in range(NCORES):
        b, g = core // 2, core % 2
        out[b, g * LROWS : (g + 1) * LROWS] = results[core]["out"]
    return out


def kernel(x, w_in, w_qkv, w_out, norm_w):
    in_maps = make_in_maps(x, w_in, w_qkv, w_out, norm_w)
    res = run_on_cores(in_maps, trace=False)
    return assemble(res.results)


if __name__ == "__main__":
    nc = build_program()
    print("program built + compiled OK")


# revision 41
# speedup vs baseline: 1.8869x; 1.1791x over previous
"""Trainium2 Bass kernel for nn_AttentionOp_60988535603899.

Linear-attention (elu+1 feature map) block:
  x_proj = x @ w_in.T ; qkv = x_proj @ w_qkv.T ; per-head linear attention
  with kv-state; raw (B,H,L,D)->(B,L,H*D) reshape; out_proj; residual; RMS norm.

Sharding: 8 cores = 4 batches x 2 head-groups (8 heads each). No collectives.

v2 design (vs baseline):
  - All projection matmuls fp8 DoubleRow (x_proj, qkv, out_proj); residual
    recompute in bf16. Scales: w_in/w_qkv/w_out/z carry x16 each; the RMS
    normalization at the end cancels the combined scale automatically.
  - The attention output is produced directly TRANSPOSED (zT[(j%2)*64+d,
    cc, m] = z[l=16m+2cc+(j%2), d]) so out_proj needs no PE transposes at
    all (the transposes poisoned HAM clock-gating in the baseline) and z
    never round-trips through DRAM.
  - The 1/normalizer is folded into qfT before the z matmul: norm row per
    head via a block-diagonal ksum matmul, reciprocal, then a PE
    broadcast-matmul (sel outer product) to spread recip across the 128
    partitions of each qfT slot.
  - kv state accumulates directly in PSUM across all 32 token chunks
    (single has_written clear at the start) - no Vector adds, which were
    stalling the PE every l-tile in the baseline and re-throttling HAM.
"""

import sys

for _p in ("/opt/trn_rl_repo",):
    if _p not in sys.path:
        sys.path.insert(0, _p)

import numpy as np

import concourse.bass as bass  # noqa: F401  (bass must import before tile)
import concourse.mybir as mybir
import concourse.tile as tile
from concourse import bacc
from concourse.bass_utils import run_bass_kernel_spmd

F32 = mybir.dt.float32
BF16 = mybir.dt.bfloat16
FP8 = mybir.dt.float8e4
ALU = mybir.AluOpType
ACTF = mybir.ActivationFunctionType
DR = mybir.MatmulPerfMode.DoubleRow

B, L, CIN, DL = 4, 4096, 512, 1024
H, DH = 16, 64
HLOC = 8                  # heads per core
ELOC = 3 * HLOC * DH      # 1536 local qkv dims
LROWS = 2048              # output rows per core
EPS = float(np.finfo(np.float32).eps)
NCORES = 8
W16 = 16.0                # fp8 weight/activation scale
WF = 32.0                 # fused qkv-weight fp8 scale

_prog_cache = {}


def _build_body(tc, xT, xresd, w_inT_bfd, w_qkvTd, w_outTd, norm_w, sel2d, out):
    nc = tc.nc

    with tc.tile_pool(name="consts", bufs=1) as consts:
        # ---------------- persistent tiles ----------------
        x_sb = consts.tile([128, 4, L], FP8, name="x_sb")
        xres = consts.tile([128, 4, LROWS], BF16, name="xres")
        w_inT_bf = consts.tile([128, 4, DL], BF16, name="w_inT_bf")
        w_qkvT = consts.tile([128, 4, ELOC], FP8, name="w_qkvT")
        w_outT = consts.tile([128, 8, DL], FP8, name="w_outT")
        qfT = consts.tile([128, 4, L], BF16, name="qfT")
        zT = consts.tile([128, HLOC, 8, 256], FP8, name="zT")
        kv_sb = consts.tile([128, 4, DH + 1], BF16, name="kv_sb")
        kv_bounce = consts.tile([64, 8, DH + 1], BF16, name="kv_bounce")
        ksb = consts.tile([128, 4, 2], BF16, name="ksb")
        sel2 = consts.tile([2, 128], mybir.dt.float32r, name="sel2")
        nw_sb = consts.tile([128, DL], F32, name="nw_sb")
        eps_sb = consts.tile([128, 1], F32, name="eps_sb")

        # ---------------- input DMAs ----------------
        # sync queue: x chunks (needed first); gpsimd: weights.
        xv = xT.rearrange("(c p) l -> p c l", p=128)
        for i in range(2):
            eng = nc.sync if i % 2 == 0 else nc.gpsimd
            eng.dma_start(x_sb[:, :, i * 1024 : (i + 1) * 1024], xv[:, :, i * 1024 : (i + 1) * 1024])
        nc.gpsimd.dma_start(w_qkvT[:], w_qkvTd.rearrange("(c p) e -> p c e", p=128))

        nc.vector.memset(eps_sb[:], 4.0 * EPS)
        # sel2[i, p] = 1 iff p//64 == i (partition-broadcast selector)
        nc.sync.dma_start(sel2[:], sel2d[:])

        # ---------------- phases 1-2: projections + kv state ----------------
        with (
            tc.tile_pool(name="w12", bufs=3) as w12,
            tc.tile_pool(name="ps_x", bufs=2, space="PSUM") as ps_x,
            tc.tile_pool(name="ps_kv", bufs=4, space="PSUM") as ps_kv,
            tc.tile_pool(name="ps_acc", bufs=1, space="PSUM") as ps_acc,
        ):
            # even heads accumulate in kv_ps_a, odd in kv_ps_b, across all
            # 32 token chunks (PSUM has_written semantics: one bank clear
            # at the start, then per-element accumulate/overwrite).
            kv_ps_a = ps_acc.tile([64, 4, DH + 1], F32, name="kv_ps_a")
            kv_ps_b = ps_acc.tile([64, 4, DH + 1], F32, name="kv_ps_b")

            xrv = xresd.rearrange("(c p) l -> p c l", p=128)
            for lt in range(8):  # 512-token tiles
                ls_l = lt * 512
                # deferred phase-3/4 loads: issued mid-loop so they don't
                # contend with the startup x/w_qkv DMA burst
                if lt == 1:
                    nc.sync.dma_start(x_sb[:, :, 2048:3072], xv[:, :, 2048:3072])
                if lt == 2:
                    nc.gpsimd.dma_start(x_sb[:, :, 3072:4096], xv[:, :, 3072:4096])
                    nc.gpsimd.dma_start(
                        w_outT[:], w_outTd.rearrange("(c p) e -> p c e", p=128)
                    )
                if lt == 3:
                    nc.gpsimd.dma_start(
                        w_inT_bf[:], w_inT_bfd.rearrange("(c p) d -> p c d", p=128)
                    )
                if lt == 5:
                    nc.sync.dma_start(
                        nw_sb[:],
                        norm_w.rearrange("(a d) -> a d", a=1).to_broadcast((128, DL)),
                    )
                if lt >= 4:
                    q = lt - 4
                    nc.sync.dma_start(
                        xres[:, :, q * 512 : (q + 1) * 512],
                        xrv[:, :, q * 512 : (q + 1) * 512],
                    )
                # q-projection (fused x @ W_f), transposed layout [dq, l]
                for qq in range(4):
                    ps = ps_x.tile([128, 512], F32, tag="mm", name="psq")
                    for c2 in range(2):
                        nc.tensor.matmul(
                            ps[:],
                            w_qkvT[:, 2 * c2 : 2 * c2 + 2, qq * 128 : (qq + 1) * 128],
                            x_sb[:, 2 * c2 : 2 * c2 + 2, ls_l : ls_l + 512],
                            start=(c2 == 0),
                            stop=(c2 == 1),
                            perf_mode=DR,
                        )
                    eq = w12.tile([128, 512], BF16, name="eq")
                    rq = w12.tile([128, 512], BF16, name="rq")
                    nc.scalar.activation(eq[:], ps[:], ACTF.Exp, scale=1.0 / WF)
                    nc.vector.tensor_scalar(rq[:], ps[:], 0.0, 1.0 / WF, ALU.max, ALU.mult)
                    # qf = min(eq, 1) + rq in one pass
                    nc.vector.scalar_tensor_tensor(
                        qfT[:, qq, ls_l : ls_l + 512], eq[:], 1.0, rq[:], ALU.min, ALU.add
                    )

                # k/v projection in [l, e] layout, 128-token subtiles
                for ls in range(4):
                    lhs = x_sb[:, :, ls_l + ls * 128 : ls_l + (ls + 1) * 128]
                    k_ps = ps_kv.tile([128, 512], F32, tag="kvp", name="k_ps")
                    v_ps = ps_kv.tile([128, 512], F32, tag="kvp", name="v_ps")
                    for c2 in range(2):
                        nc.tensor.matmul(
                            k_ps[:],
                            lhs[:, 2 * c2 : 2 * c2 + 2, :],
                            w_qkvT[:, 2 * c2 : 2 * c2 + 2, 512:1024],
                            start=(c2 == 0),
                            stop=(c2 == 1),
                            perf_mode=DR,
                        )
                    for c2 in range(2):
                        nc.tensor.matmul(
                            v_ps[:],
                            lhs[:, 2 * c2 : 2 * c2 + 2, :],
                            w_qkvT[:, 2 * c2 : 2 * c2 + 2, 1024:1536],
                            start=(c2 == 0),
                            stop=(c2 == 1),
                            perf_mode=DR,
                        )
                    kf = w12.tile([128, 512], BF16, name="kf")
                    ek = w12.tile([128, 512], BF16, name="ek")
                    nc.scalar.activation(ek[:], k_ps[:], ACTF.Exp, scale=1.0 / WF)
                    nc.vector.tensor_scalar(kf[:], k_ps[:], 0.0, 1.0 / WF, ALU.max, ALU.mult)
                    nc.vector.scalar_tensor_tensor(
                        kf[:], ek[:], 1.0, kf[:], ALU.min, ALU.add
                    )

                    vt = w12.tile([128, HLOC, DH + 1], BF16, name="vt")
                    nc.vector.tensor_scalar(
                        vt[:, :, 0:DH],
                        v_ps[:].rearrange("p (h m) -> p h m", m=DH),
                        1.0 / WF,
                        None,
                        ALU.mult,
                    )
                    nc.vector.memset(vt[:, :, DH : DH + 1], 1.0)
                    first = lt == 0 and ls == 0
                    last = lt == 7 and ls == 3
                    for h in range(HLOC):
                        nc.tensor.matmul(
                            (kv_ps_a if h % 2 == 0 else kv_ps_b)[:, h // 2, :],
                            kf[:, h * DH : (h + 1) * DH],
                            vt[:, h, :],
                            start=(first and h < 2),
                            stop=(last and h >= 6),
                            skip_group_check=True,
                        )

            # kv state: cast to bf16, partition-move odd heads to 64..127
            nc.vector.tensor_copy(kv_bounce[:, 0:4, :], kv_ps_a[:])
            nc.vector.tensor_copy(kv_bounce[:, 4:8, :], kv_ps_b[:])
            nc.sync.dma_start(kv_sb[0:64, :, :], kv_bounce[:, 0:4, :])
            nc.sync.dma_start(kv_sb[64:128, :, :], kv_bounce[:, 4:8, :])

        # ---------------- normalizer: norm rows, recip, fold into qfT ----
        # ksb[64*i+d, s, i] = ksum of head 2s+i (kv col DH), else 0.
        nc.vector.memset(ksb[:], 0.0)
        nc.vector.tensor_copy(ksb[0:64, :, 0:1], kv_sb[0:64, :, DH : DH + 1])
        nc.vector.tensor_copy(ksb[64:128, :, 1:2], kv_sb[64:128, :, DH : DH + 1])

        # ---------------- phases 3-4 + normalizer, interleaved -------------
        # recip = 1/norm via fast-NR; the x16 z-scale is folded into sel2.
        with (
            tc.tile_pool(name="nsb", bufs=2) as nsb,
            tc.tile_pool(name="p34", bufs=3) as p34,
            tc.tile_pool(name="ps_n", bufs=1, space="PSUM") as ps_n,
            tc.tile_pool(name="psm", bufs=3, space="PSUM") as psm,
            tc.tile_pool(name="ps4", bufs=2, space="PSUM") as ps4p,
        ):
            pn = ps_n.tile([34, 512], F32, name="pn")

            def norm_slot(s):
                for c in range(8):
                    cs = slice(c * 512, (c + 1) * 512)
                    pb = (c % 2) * 32  # two pn slots in one bank, 32-aligned
                    nc.tensor.matmul(
                        pn[pb : pb + 2, :], ksb[:, s, :], qfT[:, s, cs],
                        start=True, stop=True, skip_group_check=True,
                    )
                    rc = nsb.tile([2, 512], F32, name="rc")
                    nc.vector.reciprocal_approx_fast(rc[:], pn[pb : pb + 2, :])
                    rcr = nsb.tile([2, 512], mybir.dt.float32r, name="rcr")
                    nc.scalar.activation(rcr[:], rc[:], ACTF.Copy)
                    rb = psm.tile([128, 512], F32, tag="ps34", name="rb")
                    nc.tensor.matmul(rb[:], sel2[:], rcr[:], start=True, stop=True)
                    nc.vector.tensor_tensor(qfT[:, s, cs], qfT[:, s, cs], rb[:], ALU.mult)

            def phase3(h):
                s, par = h // 2, h % 2
                p0 = par * 64
                # zT[p, cc, m] = z[l=16m+2cc+(p//64), d=p%64] * 16
                for c2 in range(4):
                    ps3 = psm.tile([128, 2, 256], F32, tag="ps34", name="ps3")
                    for ci in range(2):
                        cc = 2 * c2 + ci
                        for pj in range(2):
                            nc.tensor.matmul(
                                ps3[pj * 64 : (pj + 1) * 64, ci, :],
                                kv_sb[p0 : p0 + 64, s, 0:DH],
                                qfT[p0 : p0 + 64, s, 2 * cc + pj : L : 16],
                                start=True,
                                stop=True,
                                skip_group_check=True,
                            )
                    nc.any.tensor_copy(zT[:, h, 2 * c2 : 2 * c2 + 2, :], ps3[:])

            def phase4(h):
                s, par = h // 2, h % 2
                for half in range(2):
                    lr = h * 256 + half * 128
                    ps4 = ps4p.tile([128, DL], F32, tag="p4", name="ps4")
                    for e in range(2):
                        es = slice(e * 512, (e + 1) * 512)
                        for t in range(4):
                            nc.tensor.matmul(
                                ps4[:, es],
                                zT[:, h, 2 * t : 2 * t + 2, half * 128 : (half + 1) * 128],
                                w_outT[:, 2 * t : 2 * t + 2, es],
                                start=(t == 0),
                                stop=False,
                                perf_mode=DR,
                            )
                        for cc in range(4):
                            nc.tensor.matmul(
                                ps4[:, es],
                                xres[:, cc, lr : lr + 128],
                                w_inT_bf[:, cc, es],
                                start=False,
                                stop=(cc == 3),
                            )
                    # RMS stats + final scale read PSUM directly
                    sq = p34.tile([128, DL], F32, name="sq")
                    ssum = p34.tile([128, 1], F32, name="ssum")
                    nc.scalar.activation(sq[:], ps4[:], ACTF.Square, accum_out=ssum[:])
                    # ps4 = 256*y, ssum = 2^16*sum(y^2); srt = 2*rms(y)
                    srt = p34.tile([128, 1], F32, name="srt")
                    nc.scalar.activation(
                        srt[:], ssum[:], ACTF.Sqrt, scale=1.0 / 16777216.0, bias=eps_sb[:]
                    )
                    rcp = p34.tile([128, 1], F32, name="rcp")
                    nc.vector.reciprocal_approx_fast(rcp[:], srt[:])
                    # o = (ps4 * rcp) * nw_sb  (norm_w pre-scaled by 1/128 on host)
                    o = p34.tile([128, DL], F32, name="o")
                    nc.vector.scalar_tensor_tensor(
                        o[:], ps4[:], rcp[:], nw_sb[:], ALU.mult, ALU.mult
                    )
                    eng = nc.sync if (h + half) % 2 == 0 else nc.gpsimd
                    eng.dma_start(out[lr : lr + 128, :], o[:])

            # slot-s norms feed heads 2s/2s+1; stagger so PE stays fed while
            # the recip/scale chain of the next slot drains on DVE.
            norm_slot(0)
            norm_slot(1)
            phase3(0)
            phase3(1)
            phase4(0)
            norm_slot(2)
            phase3(2)
            phase4(1)
            phase3(3)
            phase4(2)
            norm_slot(3)
            phase3(4)
            phase4(3)
            phase3(5)
            phase4(4)
            phase3(6)
            phase4(5)
            phase3(7)
            phase4(6)
            phase4(7)


def build_program():
    if "nc" in _prog_cache:
        return _prog_cache["nc"]
    nc = bacc.Bacc(None, target_bir_lowering=False, debug=False)
    xT = nc.dram_tensor("xT", [CIN, L], FP8, kind="ExternalInput")
    xresd = nc.dram_tensor("xres", [CIN, LROWS], BF16, kind="ExternalInput")
    w_inT_bfd = nc.dram_tensor("w_inT_bf", [CIN, DL], BF16, kind="ExternalInput")
    w_qkvTd = nc.dram_tensor("w_qkvT", [CIN, ELOC], FP8, kind="ExternalInput")
    w_outTd = nc.dram_tensor("w_outT", [DL, DL], FP8, kind="ExternalInput")
    norm_w = nc.dram_tensor("norm_w", [DL], F32, kind="ExternalInput")
    sel2d = nc.dram_tensor("sel2", [2, 128], mybir.dt.float32r, kind="ExternalInput")
    out = nc.dram_tensor("out", [LROWS, DL], F32, kind="ExternalOutput")
    with tile.TileContext(nc) as tc:
        _build_body(
            tc, xT[:], xresd[:], w_inT_bfd[:], w_qkvTd[:],
            w_outTd[:], norm_w[:], sel2d[:], out[:],
        )
    nc.compile()
    _prog_cache["nc"] = nc
    return nc


def make_in_maps(x, w_in, w_qkv, w_out, norm_w):
    import ml_dtypes

    bf16 = ml_dtypes.bfloat16
    f8e4 = mybir.dt.np(mybir.dt.float8e4)
    x = np.ascontiguousarray(np.asarray(x, dtype=np.float32))
    w_in = np.asarray(w_in, dtype=np.float32)
    w_qkv = np.asarray(w_qkv, dtype=np.float32)
    w_out = np.asarray(w_out, dtype=np.float32)
    norm_w = np.ascontiguousarray(np.asarray(norm_w, dtype=np.float32))

    w_inT_bf = np.ascontiguousarray(w_in.T * (W16 * W16)).astype(bf16)
    # fused qkv weight: qkv = (x @ w_in.T) @ w_qkv.T = x @ (w_qkv @ w_in).T
    w_f = w_qkv @ w_in  # [3072, 512]
    # w_outT in (token-parity, d) partition layout, cc-chunked:
    # arr[p, cc, e] = w_out.T[(2cc + p//64)*64 + p%64, e] * 16
    wt = np.ascontiguousarray(w_out.T).reshape(8, 2, 64, DL)
    w_outT = np.ascontiguousarray(
        (wt.transpose(1, 2, 0, 3).reshape(128, 8, DL) * W16)
        .transpose(1, 0, 2)
        .reshape(DL, DL)
    ).astype(f8e4)

    sel2 = np.zeros((2, 128), dtype=np.float32)
    sel2[0, 0:64] = W16   # broadcast selector, carries the x16 z-scale
    sel2[1, 64:128] = W16

    in_maps = []
    for core in range(NCORES):
        b, g = core // 2, core % 2
        sl = slice(g * 512, (g + 1) * 512)
        wq = np.concatenate(
            [w_f[0:1024][sl], w_f[1024:2048][sl], w_f[2048:3072][sl]], axis=0
        )
        in_maps.append(
            {
                "xT": np.ascontiguousarray(x[b].T).astype(f8e4),
                "xres": np.ascontiguousarray(
                    x[b, g * LROWS : (g + 1) * LROWS].T
                ).astype(bf16),
                "w_inT_bf": w_inT_bf,
                "w_qkvT": (np.ascontiguousarray(wq.T) * WF).astype(f8e4),
                "w_outT": w_outT,
                "norm_w": norm_w / 128.0,
                "sel2": sel2,
            }
        )
    return in_maps


def run_on_cores(in_maps, trace=False):
    nc = build_program()
    return run_bass_kernel_spmd(nc, in_maps, list(range(NCORES)), trace=trace)


def assemble(results):
    out = np.empty((B, L, DL), np.float32)
    for core in range(NCORES):
        b, g = core // 2, core % 2
        out[b, g * LROWS : (g + 1) * LROWS] = results[core]["out"]
    return out


def kernel(x, w_in, w_qkv, w_out, norm_w):
    in_maps = make_in_maps(x, w_in, w_qkv, w_out, norm_w)
    res = run_on_cores(in_maps, trace=False)
    return assemble(res.results)


if __name__ == "__main__":
    nc = build_program()
    print("program built + compiled OK")
